# revision 1
# baseline (speedup 1.0000x reference)
"""Trainium2 Bass kernel for nn_CustomCrossModalAttention (B=2, N=2048, D=768, H=12).

Sharding (8 cores, zero redundant matmul work):
  - core c owns batch b = c//4 and query rows [512*(c%4), 512*(c%4)+512) of that batch.
  - Phase 1 (row-parallel): each core computes q, k, v projections + LayerNorm for its
    512 rows only. k is folded with the positional term: the reference computes
    scores = (q@k^T)*scale + q@pos^T == scale * (q @ (k + pos/scale)^T), so we build
    k' = LN_k(xk) + pos/scale once.
  - Two AllGathers per 4-core batch group exchange the k'^T and v shards (k first so
    score matmuls can start while v is still in flight).
  - Phase 2 (row-parallel): 12-head attention on the core's 512 query rows with
    softmax (exp without max-subtraction; row sums via a ones column appended to v),
    then output proj, gate, fuse and final LayerNorm.

Precision: projections and the output projection run in float32r (TF32-like, same
PE throughput as bf16 at moving-dim >= 256); attention internals and the gate run
in bf16; all accumulation fp32.

Algebraic folds done on the host (all exact):
  - LN(v) gain/bias folded into wo / bo (uses sum_m attn[n,m] == 1 post-normalize).
  - q/k LN gain+bias applied during the PE-transpose copy-out (per-partition scalars
    in the transposed layout).
  - All matmul biases applied as an extra K=1 matmul with a ones row.
"""

import numpy as np
import ml_dtypes

B, N, D = 2, 2048, 768
H, DH = 12, 64
P = 128
CORES, GROUP = 8, 4
S = 512            # query rows per core
NCH = S // P       # 4 row chunks per core
MCH = N // P       # 16 key chunks
G6 = D // P        # 6
SCALE = DH ** -0.5
EPS = 1e-5

BF = ml_dtypes.bfloat16

_CACHE = {}


def _build():
    from contextlib import ExitStack

    import concourse.bacc as bacc
    import concourse.mybir as mybir
    import concourse.tile as tile
    from concourse.masks import make_identity

    f32 = mybir.dt.float32
    f32r = mybir.dt.float32r
    bf16 = mybir.dt.bfloat16
    ALU = mybir.AluOpType
    ACTF = mybir.ActivationFunctionType

    nc = bacc.Bacc("TRN2", target_bir_lowering=False, num_devices=CORES)

    def din(name, shape, dt=bf16):
        return nc.dram_tensor(name, shape, dt, kind="ExternalInput")

    xqT = din("xqT", [D, S], f32r)      # infrared rows, transposed
    xvT = din("xvT", [D, S], f32r)      # visible rows, transposed
    vis_nat = din("vis_nat", [S, D], f32)
    posTb = din("posTb", [D, S])        # pos/scale + lnk_b, transposed (bf16)
    wqkvT = din("wqkvT", [D, 3 * D], f32r)
    bqkv = din("bqkv", [1, 3 * D], f32r)
    woT = din("woT", [D, D], f32r)      # (wo * lnv_w).T
    bo_a = din("bo_a", [1, D], f32r)    # bo + wo @ lnv_b
    gwT = din("gwT", [2 * D, D], f32r)
    gb = din("gb", [1, D], f32r)
    lnq_g = din("lnq_g", [P, G6], f32)
    lnq_b = din("lnq_b", [P, G6], f32)
    lnk_g = din("lnk_g", [P, G6], f32)
    lnf = din("lnf", [2, D], f32)
    out_rows = nc.dram_tensor("out_rows", [S, D], f32, kind="ExternalOutput")

    FLK = D * S                      # k'^T payload
    FLV = NCH * P * H * (DH + 1)     # v payload (padded with ones col)
    cc_in_k = nc.dram_tensor("cc_in_k", [FLK], f32r)
    cc_out_k = nc.dram_tensor("cc_out_k", [GROUP, FLK], f32r)
    cc_in_v = nc.dram_tensor("cc_in_v", [FLV], bf16)
    cc_out_v = nc.dram_tensor("cc_out_v", [GROUP, FLV], bf16)
    groups = [[0, 1, 2, 3], [4, 5, 6, 7]]

    HALves = [(0, 512), (512, D)]

    with tile.TileContext(nc) as tc, ExitStack() as ctx:
        const = ctx.enter_context(tc.tile_pool(name="const", bufs=1))
        persist = ctx.enter_context(tc.tile_pool(name="persist", bufs=1))

        ident = const.tile([P, P], bf16)
        make_identity(nc, ident)
        ident_f32 = const.tile([P, P], f32)
        make_identity(nc, ident_f32)
        ones_r_f = const.tile([1, P], f32)
        nc.vector.memset(ones_r_f, 1.0)
        ones_r = ones_r_f.bitcast(f32r)
        ones_bf = const.tile([1, P], bf16)
        nc.vector.memset(ones_bf, 1.0)
        ones_f32 = const.tile([1, P], f32)
        nc.vector.memset(ones_f32, 1.0)
        eps_t = const.tile([P, 1], f32)
        nc.vector.memset(eps_t, EPS)

        xvT_sb = const.tile([P, G6, S], f32r)
        nc.gpsimd.dma_start(out=xvT_sb, in_=xvT.rearrange("(s p) n -> p s n", p=P))
        woT_sb = const.tile([P, G6, D], f32r)
        nc.scalar.dma_start(out=woT_sb, in_=woT.rearrange("(s p) o -> p s o", p=P))
        bo_sb = const.tile([1, D], f32r)
        nc.sync.dma_start(out=bo_sb, in_=bo_a.ap())
        gb_sb = const.tile([1, D], f32r)
        nc.sync.dma_start(out=gb_sb, in_=gb.ap())
        lnq_g_sb = const.tile([P, G6], f32)
        nc.sync.dma_start(out=lnq_g_sb, in_=lnq_g.ap())
        lnq_b_sb = const.tile([P, G6], f32)
        nc.sync.dma_start(out=lnq_b_sb, in_=lnq_b.ap())
        lnk_g_sb = const.tile([P, G6], f32)
        nc.sync.dma_start(out=lnk_g_sb, in_=lnk_g.ap())
        lnfw_sb = const.tile([1, D], f32)
        nc.sync.dma_start(out=lnfw_sb, in_=lnf.ap()[0:1, :])
        lnfb_sb = const.tile([1, D], f32)
        nc.sync.dma_start(out=lnfb_sb, in_=lnf.ap()[1:2, :])

        outT_sb = persist.tile([P, G6, S], f32r)

        with tc.tile_pool(name="mid", bufs=1) as midp:
            qT_sb = midp.tile([P, G6, S], f32r)

            with (
                tc.tile_pool(name="ph1", bufs=1) as ph1,
                tc.tile_pool(name="wrot", bufs=2) as wrot,
                tc.tile_pool(name="pwork", bufs=1) as pwork,
                tc.tile_pool(name="stat", bufs=6) as stat,
                tc.tile_pool(name="psum_p", bufs=2, space="PSUM") as psum_p,
                tc.tile_pool(name="psum_t", bufs=2, space="PSUM") as psum_t,
            ):
                xqT_sb = ph1.tile([P, G6, S], f32r)
                nc.gpsimd.dma_start(
                    out=xqT_sb, in_=xqT.rearrange("(s p) n -> p s n", p=P)
                )
                bqkv_sb = ph1.tile([1, 3 * D], f32r)
                nc.sync.dma_start(out=bqkv_sb, in_=bqkv.ap())
                posTb_sb = ph1.tile([P, G6, S], bf16)
                nc.sync.dma_start(
                    out=posTb_sb, in_=posTb.rearrange("(s p) n -> p s n", p=P)
                )

                kloc_sb = ph1.tile([P, G6, S], f32r)   # local k'^T shard
                vloc_sb = ph1.tile([P, NCH, H, DH + 1], bf16)
                nc.vector.memset(vloc_sb[:, :, :, DH:DH + 1], 1.0)

                def load_w(off):
                    w_sb = wrot.tile([P, G6, D], f32r, tag="w")
                    nc.sync.dma_start(
                        out=w_sb,
                        in_=wqkvT.rearrange("(s p) o -> p s o", p=P)[:, :, off:off + D],
                    )
                    return w_sb

                def proj_tile(lhsT_sb, w_sb, w_off, c):
                    py = psum_p.tile([P, D], f32)
                    for o0, o1 in HALves:
                        for s in range(G6):
                            nc.tensor.matmul(
                                py[:, o0:o1],
                                lhsT_sb[:, s, c * P:(c + 1) * P],
                                w_sb[:, s, o0:o1],
                                start=(s == 0), stop=False,
                            )
                        nc.tensor.matmul(
                            py[:, o0:o1], ones_r,
                            bqkv_sb[:, w_off + o0:w_off + o1],
                            start=False, stop=True,
                        )
                    return py

                def ln_stats(y, pool):
                    st = pool.tile([P, 2, 6], f32)
                    for i in range(2):
                        nc.vector.bn_stats(
                            out=st[:, i], in_=y[:, i * 384:(i + 1) * 384]
                        )
                    mv = pool.tile([P, 2], f32)
                    nc.vector.bn_aggr(out=mv, in_=st)
                    rstd = pool.tile([P, 1], f32)
                    nc.scalar.activation(
                        out=rstd, in_=mv[:, 1:2], func=ACTF.Sqrt,
                        bias=eps_t, scale=1.0,
                    )
                    nc.vector.reciprocal(out=rstd, in_=rstd)
                    # negmr = -mu*rstd: ACT applies (y-mu)*rstd as y*rstd+negmr
                    negmr = pool.tile([P, 1], f32)
                    nc.vector.tensor_scalar(
                        out=negmr, in0=mv[:, 0:1], scalar1=rstd, scalar2=-1.0,
                        op0=ALU.mult, op1=ALU.mult,
                    )
                    return negmr, rstd

                # ---- k' ----
                wk_sb = load_w(D)
                knats = []
                for c in range(NCH):
                    py = proj_tile(xvT_sb, wk_sb, D, c)
                    negmr, rstd = ln_stats(py, stat)
                    knat = pwork.tile([P, D], f32, tag=f"knat{c}")
                    nc.scalar.activation(
                        out=knat, in_=py, func=ACTF.Identity,
                        bias=negmr, scale=rstd,
                    )
                    knats.append(knat)
                for s in range(G6):
                    pt = psum_t.tile([P, NCH, P], f32)
                    for c in range(NCH):
                        nc.tensor.transpose(
                            pt[:, c], knats[c][:, s * P:(s + 1) * P], ident_f32
                        )
                    nc.vector.scalar_tensor_tensor(
                        out=kloc_sb[:, s, :],
                        in0=pt.rearrange("p c n -> p (c n)"),
                        scalar=lnk_g_sb[:, s:s + 1],
                        in1=posTb_sb[:, s, :],
                        op0=ALU.mult, op1=ALU.add,
                    )

                # ---- exchange k' (scores need it first) ----
                nc.sync.dma_start(
                    out=cc_in_k.ap().rearrange("(s p n) -> p s n", p=P, s=G6),
                    in_=kloc_sb,
                )
                nc.gpsimd.collective_compute(
                    "AllGather", ALU.bypass, replica_groups=groups,
                    ins=[cc_in_k.ap().opt()], outs=[cc_out_k.ap().opt()],
                )
                # ---- v ----
                wv_sb = load_w(2 * D)
                for c in range(NCH):
                    py = proj_tile(xvT_sb, wv_sb, 2 * D, c)
                    negmr, rstd = ln_stats(py, stat)
                    nc.scalar.activation(
                        out=vloc_sb[:, c, :, 0:DH],
                        in_=py.rearrange("p (h d) -> p h d", h=H),
                        func=ACTF.Identity, bias=negmr, scale=rstd,
                    )

                nc.sync.dma_start(
                    out=cc_in_v.ap().rearrange("(c p f) -> p c f", c=NCH, p=P),
                    in_=vloc_sb.rearrange("p c h d -> p c (h d)"),
                )
                nc.gpsimd.collective_compute(
                    "AllGather", ALU.bypass, replica_groups=groups,
                    ins=[cc_in_v.ap().opt()], outs=[cc_out_v.ap().opt()],
                )
                # ---- q ----
                wq_sb = load_w(0)
                qnats = []
                for c in range(NCH):
                    py = proj_tile(xqT_sb, wq_sb, 0, c)
                    negmr, rstd = ln_stats(py, stat)
                    qnat = pwork.tile([P, D], f32, tag=f"qnat{c}")
                    nc.scalar.activation(
                        out=qnat, in_=py, func=ACTF.Identity,
                        bias=negmr, scale=rstd,
                    )
                    qnats.append(qnat)
                for s in range(G6):
                    pt = psum_t.tile([P, NCH, P], f32)
                    for c in range(NCH):
                        nc.tensor.transpose(
                            pt[:, c], qnats[c][:, s * P:(s + 1) * P], ident_f32
                        )
                    nc.vector.scalar_tensor_tensor(
                        out=qT_sb[:, s, :],
                        in0=pt.rearrange("p c n -> p (c n)"),
                        scalar=lnq_g_sb[:, s:s + 1],
                        in1=lnq_b_sb[:, s:s + 1].to_broadcast([P, S]),
                        op0=ALU.mult, op1=ALU.add,
                    )

            # ---- attention ----
            with (
                tc.tile_pool(name="gath", bufs=1) as gath,
                tc.tile_pool(name="attn", bufs=3) as apool,
                tc.tile_pool(name="hwork", bufs=4) as hwork,
                tc.tile_pool(name="ps_s", bufs=2, space="PSUM") as ps_s,
                tc.tile_pool(name="ps_o", bufs=2, space="PSUM") as ps_o,
            ):
                kT_sb = gath.tile([P, G6, GROUP, S], f32r)      # gathered k'^T
                vaug_sb = gath.tile([P, MCH, H, DH + 1], bf16)  # gathered v + ones
                for g in range(GROUP):
                    eng = nc.sync if g % 2 == 0 else nc.scalar
                    eng.dma_start(
                        out=kT_sb[:, :, g, :],
                        in_=cc_out_k[g:g + 1, :].rearrange(
                            "x (s p n) -> (x p) s n", p=P, s=G6
                        ),
                    )
                for g in range(GROUP):
                    eng = nc.sync if g % 2 == 0 else nc.scalar
                    eng.dma_start(
                        out=vaug_sb[:, 4 * g:4 * g + 4, :, :].rearrange(
                            "p c h d -> p c (h d)"
                        ),
                        in_=cc_out_v[g:g + 1, :].rearrange(
                            "x (c p f) -> (x p) c f", c=NCH, p=P
                        ),
                    )
                for h in range(H):
                    p0 = DH * (h % 2)
                    grp = h // 2
                    po = ps_o.tile([DH + 1, S], f32)
                    for mc0, w in ((0, 3), (3, 3), (6, 3), (9, 3), (12, 3), (15, 1)):
                        ps = ps_s.tile([P, 3, S], f32, tag="ps3")
                        for j in range(w):
                            mc = mc0 + j
                            nc.tensor.matmul(
                                ps[:, j],
                                kT_sb[p0:p0 + DH, grp, mc // 4,
                                      (mc % 4) * P:(mc % 4 + 1) * P],
                                qT_sb[p0:p0 + DH, grp, :],
                                start=True, stop=True,
                            )
                        at = apool.tile([P, 3, S], bf16, tag="at")
                        nc.scalar.activation(
                            out=at[:, :w], in_=ps[:, :w], func=ACTF.Exp, scale=SCALE
                        )
                        for j in range(w):
                            mc = mc0 + j
                            nc.tensor.matmul(
                                po, vaug_sb[:, mc, h, :], at[:, j],
                                start=(mc == 0), stop=(mc == MCH - 1),
                            )
                    rinv = hwork.tile([1, S], f32, tag="rinv")
                    nc.vector.reciprocal(out=rinv, in_=po[DH:DH + 1, :])
                    rbc = hwork.tile([DH, S], f32, tag="rbc")
                    nc.gpsimd.partition_broadcast(rbc, rinv)
                    nc.vector.tensor_tensor(
                        out=outT_sb[p0:p0 + DH, grp, :], in0=po[0:DH, :],
                        in1=rbc, op=ALU.mult,
                    )

        # ---- output proj, gate, fuse, final LN ----
        with (
            tc.tile_pool(name="zpool", bufs=1) as zpool,
            tc.tile_pool(name="fwork", bufs=2) as fwork,
            tc.tile_pool(name="stat2", bufs=6) as stat2,
            tc.tile_pool(name="ps_z", bufs=2, space="PSUM") as ps_z,
            tc.tile_pool(name="ps_t2", bufs=2, space="PSUM") as ps_t2,
        ):
            vis_sb = zpool.tile([P, NCH, D], f32)
            nc.gpsimd.dma_start(
                out=vis_sb, in_=vis_nat.rearrange("(c p) o -> p c o", p=P)
            )
            gwT_sb = zpool.tile([P, 2 * G6, D], f32r)
            nc.scalar.dma_start(
                out=gwT_sb, in_=gwT.rearrange("(s p) o -> p s o", p=P)
            )
            z_sb = zpool.tile([P, NCH, D], f32)
            zT_sb = zpool.tile([P, G6, S], f32r)
            gbc = zpool.tile([P, D], f32)
            bbc = zpool.tile([P, D], f32)

            # broadcast final-LN gain/bias across partitions via K=1 matmul
            for dst, src_row in ((gbc, lnfw_sb), (bbc, lnfb_sb)):
                pb = ps_z.tile([P, D], f32, tag="pz")
                for o0, o1 in HALves:
                    nc.tensor.matmul(
                        pb[:, o0:o1], ones_f32, src_row[:, o0:o1],
                        start=True, stop=True,
                    )
                nc.vector.tensor_copy(out=dst, in_=pb)

            def ln_stats2(y):
                st = stat2.tile([P, 2, 6], f32)
                for i in range(2):
                    nc.vector.bn_stats(out=st[:, i], in_=y[:, i * 384:(i + 1) * 384])
                mv = stat2.tile([P, 2], f32)
                nc.vector.bn_aggr(out=mv, in_=st)
                rstd = stat2.tile([P, 1], f32)
                nc.scalar.activation(
                    out=rstd, in_=mv[:, 1:2], func=ACTF.Sqrt, bias=eps_t, scale=1.0
                )
                nc.vector.reciprocal(out=rstd, in_=rstd)
                negmr = stat2.tile([P, 1], f32)
                nc.vector.tensor_scalar(
                    out=negmr, in0=mv[:, 0:1], scalar1=rstd, scalar2=-1.0,
                    op0=ALU.mult, op1=ALU.mult,
                )
                return negmr, rstd

            for c in range(NCH):
                pz = ps_z.tile([P, D], f32, tag="pz")
                for o0, o1 in HALves:
                    for s in range(G6):
                        nc.tensor.matmul(
                            pz[:, o0:o1],
                            outT_sb[:, s, c * P:(c + 1) * P],
                            woT_sb[:, s, o0:o1],
                            start=(s == 0), stop=False,
                        )
                    nc.tensor.matmul(
                        pz[:, o0:o1], ones_r, bo_sb[:, o0:o1],
                        start=False, stop=True,
                    )
                nc.scalar.copy(out=z_sb[:, c], in_=pz)
            for s in range(G6):
                pt = ps_t2.tile([P, NCH, P], f32)
                for c in range(NCH):
                    nc.tensor.transpose(
                        pt[:, c], z_sb[:, c, s * P:(s + 1) * P], ident_f32
                    )
                nc.scalar.copy(
                    out=zT_sb[:, s, :], in_=pt.rearrange("p c n -> p (c n)")
                )

            gsigs = []
            for c in range(NCH):
                pg = ps_z.tile([P, D], f32, tag="pz")
                for o0, o1 in HALves:
                    for s in range(G6):
                        nc.tensor.matmul(
                            pg[:, o0:o1],
                            xvT_sb[:, s, c * P:(c + 1) * P],
                            gwT_sb[:, s, o0:o1],
                            start=(s == 0), stop=False,
                        )
                    for s in range(G6):
                        nc.tensor.matmul(
                            pg[:, o0:o1],
                            zT_sb[:, s, c * P:(c + 1) * P],
                            gwT_sb[:, G6 + s, o0:o1],
                            start=False, stop=False,
                        )
                    nc.tensor.matmul(
                        pg[:, o0:o1], ones_r, gb_sb[:, o0:o1],
                        start=False, stop=True,
                    )
                gsig = zpool.tile([P, D], bf16, tag=f"gsig{c}")
                nc.scalar.activation(out=gsig, in_=pg, func=ACTF.Sigmoid)
                gsigs.append(gsig)

            for c in range(NCH):
                gsig = gsigs[c]
                dvz = fwork.tile([P, D], f32, tag="dvz")
                nc.gpsimd.tensor_tensor(
                    out=dvz, in0=vis_sb[:, c], in1=z_sb[:, c], op=ALU.subtract
                )
                fus = fwork.tile([P, D], f32, tag="fus")
                nc.vector.tensor_tensor(out=fus, in0=gsig, in1=dvz, op=ALU.mult)
                nc.vector.tensor_tensor(out=fus, in0=fus, in1=z_sb[:, c], op=ALU.add)
                negmr, rstd = ln_stats2(fus)
                tnorm = fwork.tile([P, D], f32, tag="tnorm")
                nc.scalar.activation(
                    out=tnorm, in_=fus, func=ACTF.Identity, bias=negmr, scale=rstd
                )
                nc.vector.tensor_tensor(out=tnorm, in0=tnorm, in1=gbc, op=ALU.mult)
                nc.vector.tensor_tensor(out=tnorm, in0=tnorm, in1=bbc, op=ALU.add)
                nc.sync.dma_start(
                    out=out_rows.rearrange("(c p) o -> p c o", p=P)[:, c], in_=tnorm
                )

    nc.compile()
    return nc


def _prepare_in_maps(inputs):
    f32 = np.float32
    vis = np.asarray(inputs["visible_features"], f32)
    inf = np.asarray(inputs["infrared_features"], f32)
    wq = np.asarray(inputs["wq"], f32)
    bq = np.asarray(inputs["bq"], f32)
    lnq_w = np.asarray(inputs["lnq_w"], f32)
    lnq_b = np.asarray(inputs["lnq_b"], f32)
    wk = np.asarray(inputs["wk"], f32)
    bk = np.asarray(inputs["bk"], f32)
    lnk_w = np.asarray(inputs["lnk_w"], f32)
    lnk_b = np.asarray(inputs["lnk_b"], f32)
    wv = np.asarray(inputs["wv"], f32)
    bv = np.asarray(inputs["bv"], f32)
    lnv_w = np.asarray(inputs["lnv_w"], f32)
    lnv_b = np.asarray(inputs["lnv_b"], f32)
    pos = np.asarray(inputs["pos_emb"], f32)[:N]
    wo = np.asarray(inputs["wo"], f32)
    bo = np.asarray(inputs["bo"], f32)
    gw = np.asarray(inputs["gate_w"], f32)
    gb_ = np.asarray(inputs["gate_b"], f32)
    ln_w = np.asarray(inputs["ln_w"], f32)
    ln_b = np.asarray(inputs["ln_b"], f32)

    wqkvT = np.ascontiguousarray(np.concatenate([wq.T, wk.T, wv.T], axis=1))
    bqkv = np.ascontiguousarray(np.concatenate([bq, bk, bv])[None])
    woT = np.ascontiguousarray((wo * lnv_w[None, :]).T)   # fold LN_v gain
    bo_a = np.ascontiguousarray((bo + wo @ lnv_b)[None])  # fold LN_v bias
    gwT = np.ascontiguousarray(gw.T)
    gbr = np.ascontiguousarray(gb_[None])
    lnq_g = np.ascontiguousarray(lnq_w.reshape(G6, P).T)
    lnq_b2 = np.ascontiguousarray(lnq_b.reshape(G6, P).T)
    lnk_g = np.ascontiguousarray(lnk_w.reshape(G6, P).T)
    lnf = np.stack([ln_w, ln_b])

    in_maps = []
    for c in range(CORES):
        b, r0 = c // GROUP, (c % GROUP) * S
        in_maps.append({
            "xqT": np.ascontiguousarray(inf[b, r0:r0 + S].T),
            "xvT": np.ascontiguousarray(vis[b, r0:r0 + S].T),
            "vis_nat": np.ascontiguousarray(vis[b, r0:r0 + S]),
            "posTb": np.ascontiguousarray(
                pos[r0:r0 + S].T / SCALE + lnk_b[:, None]
            ).astype(BF),
            "wqkvT": wqkvT,
            "bqkv": bqkv,
            "woT": woT,
            "bo_a": bo_a,
            "gwT": gwT,
            "gb": gbr,
            "lnq_g": lnq_g,
            "lnq_b": lnq_b2,
            "lnk_g": lnk_g,
            "lnf": lnf,
        })
    return in_maps


def kernel(trace=False, **inputs):
    from concourse.bass_utils import run_bass_kernel_spmd

    if "nc" not in _CACHE:
        _CACHE["nc"] = _build()
    nc = _CACHE["nc"]
    in_maps = _prepare_in_maps(inputs)
    res = run_bass_kernel_spmd(
        nc, in_maps, core_ids=list(range(CORES)), trace=trace
    )
    out = np.empty((B, N, D), np.float32)
    for c in range(CORES):
        b, r0 = c // GROUP, (c % GROUP) * S
        out[b, r0:r0 + S] = res.results[c]["out_rows"]
    _CACHE["last_result"] = res
    return out



# revision 9
# speedup vs baseline: 1.7371x; 1.7371x over previous
"""Trainium2 Bass kernel for nn_CustomCrossModalAttention (B=2, N=2048, D=768, H=12).

Sharding (8 cores, collective-free):
  - core c owns batch b = c//4 and query rows [512*(c%4), 512*(c%4)+512).
  - k'/v are computed REDUNDANTLY for the whole batch on each of its 4 cores
    (~45us extra PE) instead of exchanging shards: the AllGather pair cost far
    more than the replicated matmuls and serialized the whole pipeline.
  - Keys are column-PERMUTED per core so the core's own 512 rows come first
    (softmax sums over all keys, so key order is irrelevant); this makes the
    SPMD program uniform while the gate still reads "own" xv columns at a
    fixed offset 0.

Math folds (exact):
  - scores = (q@k^T)*scale + q@pos^T == scale * (q @ (LNk*g + lnk_b + pos/scale)^T)
  - LN_v gain/bias folded into wo/bo.
  - gate z-half folded through the output projection: gate = sigmoid(
      vis@gwv^T + attnout@(gwz@wo_eff)^T + gb + gwz@bo_a), removing the
    z -> zT transposes and the serialization on z.
  - All additive biases in this problem are structurally zero
    (setup_inputs uses jnp.zeros); nonzero biases are supported via
    ones-row matmuls compiled on demand (flags in the build cache key).

Dtypes (validated by numpy emulation to rel-err ~8e-3, same as the old
AllGather kernel): q path f32r end-to-end (q errors multiply the large q@pos
term in the exp argument, so bf16 there would cost ~2% at-error); k/v/gate/out
paths bf16; kT kept f32 (magnitude ~8 after the pos fold); exp/softmax in
bf16; all matmul accumulation f32 in PSUM.

Schedule: V-proj -> Q-proj -> K-proj -> per-head-pair [kT transpose block ->
attention heads 2s,2s+1] -> out-proj/gate/fuse/final-LN. Interleaving the kT
blocks with attention lets the Activation engine's softmax exp (~95us, the
2nd-busiest engine) start while PE is still projecting.
"""

import numpy as np
import ml_dtypes

B, N, D = 2, 2048, 768
H, DH = 12, 64
P = 128
CORES, GROUP = 8, 4
S = 512            # query rows per core
NCH = S // P       # 4 row chunks per core
MCH = N // P       # 16 key chunks
G6 = D // P        # 6
SCALE = DH ** -0.5
EPS = 1e-5

BF = ml_dtypes.bfloat16

_CACHE = {}

HALVES = [(0, 512), (512, D)]


def _build(has_qkv_bias, has_o_bias, has_g_bias, has_f_affine):
    from contextlib import ExitStack

    import concourse.bacc as bacc
    import concourse.mybir as mybir
    import concourse.tile as tile
    from concourse.masks import make_identity

    f32 = mybir.dt.float32
    f32r = mybir.dt.float32r
    bf16 = mybir.dt.bfloat16
    ALU = mybir.AluOpType
    ACTF = mybir.ActivationFunctionType

    nc = bacc.Bacc("TRN2", target_bir_lowering=False, num_devices=CORES)

    def din(name, shape, dt=bf16):
        return nc.dram_tensor(name, shape, dt, kind="ExternalInput")

    xqT = din("xqT", [D, S], f32r)        # own infrared rows, transposed
    xvT = din("xvT", [D, N], bf16)        # full-batch visible, transposed, key-permuted
    vis_nat = din("vis_nat", [S, D], f32)  # own visible rows, natural
    posTb = din("posTb", [D, N], bf16)    # pos/scale + lnk_b, transposed, permuted
    wqT = din("wqT", [D, D], f32r)
    wkvT = din("wkvT", [D, 2 * D], bf16)  # [wk.T | wv.T]
    woT = din("woT", [D, D], bf16)        # (wo * lnv_w).T
    gwvT = din("gwvT", [D, D], bf16)      # gate vis-half weights, transposed
    gwzT = din("gwzT", [D, D], bf16)      # (gwz @ wo_eff).T
    lnq_g = din("lnq_g", [P, G6], f32)
    lnq_b = din("lnq_b", [P, G6], f32)
    lnk_g = din("lnk_g", [P, G6], f32)
    bqkv = din("bqkv", [1, 3 * D], f32r)  # only read when has_qkv_bias
    bo_a = din("bo_a", [1, D], bf16)      # bo + wo@lnv_b
    gb_e = din("gb_e", [1, D], bf16)      # gate_b + gwz@bo_a
    lnf = din("lnf", [2, D], f32)
    out_rows = nc.dram_tensor("out_rows", [S, D], f32, kind="ExternalOutput")

    with tile.TileContext(nc) as tc, ExitStack() as ctx:
        const = ctx.enter_context(tc.tile_pool(name="const", bufs=1))
        persist = ctx.enter_context(tc.tile_pool(name="persist", bufs=1))

        ident_bf = const.tile([P, P], bf16)
        make_identity(nc, ident_bf)
        ident_f32 = const.tile([P, P], f32)
        make_identity(nc, ident_f32)
        ones_r_f = const.tile([1, P], f32)
        nc.vector.memset(ones_r_f, 1.0)
        ones_r = ones_r_f.bitcast(f32r)
        ones_b = const.tile([1, P], bf16)
        nc.vector.memset(ones_b, 1.0)
        eps_t = const.tile([P, 1], f32)
        nc.vector.memset(eps_t, EPS)

        # ---- persistent tiles ----
        xvT_sb = persist.tile([P, G6, N], bf16)
        kT_sb = persist.tile([P, G6, N], f32r)
        vaug_sb = persist.tile([P, MCH, H, DH + 1], bf16)
        qT_sb = persist.tile([P, G6, S], f32r)
        outT_sb = persist.tile([P, G6, S], bf16)
        lnq_g_sb = persist.tile([P, G6], f32)
        lnq_b_sb = persist.tile([P, G6], f32)
        lnk_g_sb = persist.tile([P, G6], f32)
        lnfw_sb = persist.tile([1, D], f32)
        lnfb_sb = persist.tile([1, D], f32)

        nc.vector.memset(vaug_sb[:, :, :, DH:DH + 1], 1.0)
        # xvT in 4 column-chunks so V-proj chunk 0 starts early
        for i in range(4):
            nc.gpsimd.dma_start(
                out=xvT_sb[:, :, i * S:(i + 1) * S],
                in_=xvT.rearrange("(s p) n -> p s n", p=P)[:, :, i * S:(i + 1) * S],
            )
        nc.sync.dma_start(out=lnq_g_sb, in_=lnq_g.ap())
        nc.sync.dma_start(out=lnq_b_sb, in_=lnq_b.ap())
        nc.sync.dma_start(out=lnk_g_sb, in_=lnk_g.ap())
        nc.sync.dma_start(out=lnfw_sb, in_=lnf.ap()[0:1, :])
        nc.sync.dma_start(out=lnfb_sb, in_=lnf.ap()[1:2, :])

        def ln_stats(y, pool):
            st = pool.tile([P, 2, 6], f32, tag="st")
            for i in range(2):
                nc.vector.bn_stats(out=st[:, i], in_=y[:, i * 384:(i + 1) * 384])
            mv = pool.tile([P, 2], f32, tag="mv")
            nc.vector.bn_aggr(out=mv, in_=st)
            rstd = pool.tile([P, 1], f32, tag="rstd")
            nc.scalar.activation(
                out=rstd, in_=mv[:, 1:2], func=ACTF.Sqrt, bias=eps_t, scale=1.0
            )
            nc.vector.reciprocal(out=rstd, in_=rstd)
            negmr = pool.tile([P, 1], f32, tag="negmr")
            nc.vector.tensor_scalar(
                out=negmr, in0=mv[:, 0:1], scalar1=rstd, scalar2=-1.0,
                op0=ALU.mult, op1=ALU.mult,
            )
            return negmr, rstd

        # ================= phase V + Q (scoped) =================
        with (
            tc.tile_pool(name="pvq", bufs=1) as pvq,
            tc.tile_pool(name="wrot", bufs=2) as wrot,
            tc.tile_pool(name="stat", bufs=4) as stat,
            tc.tile_pool(name="qn", bufs=1) as qn,
            tc.tile_pool(name="ps_p", bufs=2, space="PSUM") as ps_p,
            tc.tile_pool(name="ps_t", bufs=2, space="PSUM") as ps_t,
        ):
            wv_sb = wrot.tile([P, G6, D], bf16, tag="wkv")
            nc.sync.dma_start(
                out=wv_sb,
                in_=wkvT.rearrange("(s p) o -> p s o", p=P)[:, :, D:2 * D],
            )
            bqkv_sb = None
            if has_qkv_bias:
                bqkv_sb = pvq.tile([1, 3 * D], f32r)
                nc.sync.dma_start(out=bqkv_sb, in_=bqkv.ap())

            def proj_tile(lhsT_sb, w_sb, w_off, c):
                py = ps_p.tile([P, D], f32, tag="py")
                for o0, o1 in HALVES:
                    for s in range(G6):
                        nc.tensor.matmul(
                            py[:, o0:o1],
                            lhsT_sb[:, s, c * P:(c + 1) * P],
                            w_sb[:, s, o0:o1],
                            start=(s == 0), stop=(not has_qkv_bias and s == G6 - 1),
                        )
                    if has_qkv_bias:
                        nc.tensor.matmul(
                            py[:, o0:o1], ones_r,
                            bqkv_sb[:, w_off + o0:w_off + o1],
                            start=False, stop=True,
                        )
                return py

            # ---- V: 16 chunks, straight into vaug ----
            for c in range(MCH):
                py = proj_tile(xvT_sb, wv_sb, 2 * D, c)
                negmr, rstd = ln_stats(py, stat)
                nc.scalar.activation(
                    out=vaug_sb[:, c, :, 0:DH],
                    in_=py.rearrange("p (h d) -> p h d", h=H),
                    func=ACTF.Identity, bias=negmr, scale=rstd,
                )

            # ---- Q: 4 chunks ----
            xqT_sb = pvq.tile([P, G6, S], f32r)
            nc.scalar.dma_start(
                out=xqT_sb, in_=xqT.rearrange("(s p) n -> p s n", p=P)
            )
            wq_sb = pvq.tile([P, G6, D], f32r)
            nc.sync.dma_start(
                out=wq_sb, in_=wqT.rearrange("(s p) o -> p s o", p=P)
            )
            qnats = []
            for c in range(NCH):
                py = proj_tile(xqT_sb, wq_sb, 0, c)
                negmr, rstd = ln_stats(py, stat)
                qnat = qn.tile([P, D], f32, tag=f"qnat{c}")
                nc.scalar.activation(
                    out=qnat, in_=py, func=ACTF.Identity, bias=negmr, scale=rstd
                )
                qnats.append(qnat)
            for s in range(G6):
                pt = ps_t.tile([P, NCH, P], f32, tag="pt")
                for c in range(NCH):
                    nc.tensor.transpose(
                        pt[:, c], qnats[c][:, s * P:(s + 1) * P], ident_f32
                    )
                nc.vector.tensor_scalar(
                    out=qT_sb[:, s, :],
                    in0=pt.rearrange("p c n -> p (c n)"),
                    scalar1=lnq_g_sb[:, s:s + 1],
                    scalar2=lnq_b_sb[:, s:s + 1],
                    op0=ALU.mult, op1=ALU.add,
                )

        # ============ phase K + attention (interleaved) ============
        with (
            tc.tile_pool(name="kp", bufs=1) as kp,
            tc.tile_pool(name="wrot2", bufs=1) as wrot2,
            tc.tile_pool(name="post", bufs=2) as postp,
            tc.tile_pool(name="stat2", bufs=4) as stat2,
            tc.tile_pool(name="attn", bufs=3) as apool,
            tc.tile_pool(name="hwork", bufs=4) as hwork,
            tc.tile_pool(name="ps_s", bufs=2, space="PSUM") as ps_s,
            tc.tile_pool(name="ps_o", bufs=2, space="PSUM") as ps_o,
        ):
            wk_sb = wrot2.tile([P, G6, D], bf16)
            nc.sync.dma_start(
                out=wk_sb,
                in_=wkvT.rearrange("(s p) o -> p s o", p=P)[:, :, 0:D],
            )
            knat_sb = kp.tile([P, MCH, D], bf16)

            for c in range(MCH):
                # share the score-psum slots (tag "ps3"): K-proj, kT
                # transposes and score groups are sequential PE outputs
                py = ps_s.tile(
                    [P, 3, S], f32, tag="ps3", name="pyk"
                ).rearrange("p c n -> p (c n)")[:, 0:D]
                for o0, o1 in HALVES:
                    for s in range(G6):
                        nc.tensor.matmul(
                            py[:, o0:o1],
                            xvT_sb[:, s, c * P:(c + 1) * P],
                            wk_sb[:, s, o0:o1],
                            start=(s == 0), stop=(not has_qkv_bias and s == G6 - 1),
                        )
                    if has_qkv_bias:
                        nc.tensor.matmul(
                            py[:, o0:o1], ones_r, bqkv_sb[:, D + o0:D + o1],
                            start=False, stop=True,
                        )
                negmr, rstd = ln_stats(py, stat2)
                nc.scalar.activation(
                    out=knat_sb[:, c, :], in_=py, func=ACTF.Identity,
                    bias=negmr, scale=rstd,
                )

            # per head-pair s: build kT block s, then run heads 2s, 2s+1
            for s in range(G6):
                posT_s = postp.tile([P, N], bf16, tag="posT")
                nc.scalar.dma_start(
                    out=posT_s,
                    in_=posTb.rearrange("(s p) n -> p s n", p=P)[:, s, :],
                )
                for half in range(2):
                    pt = ps_s.tile([P, MCH // 2, P], bf16, tag="ps3")
                    for c in range(MCH // 2):
                        mc = half * 8 + c
                        nc.tensor.transpose(
                            pt[:, c], knat_sb[:, mc, s * P:(s + 1) * P], ident_bf
                        )
                    nc.vector.scalar_tensor_tensor(
                        out=kT_sb[:, s, half * 1024:(half + 1) * 1024],
                        in0=pt.rearrange("p c n -> p (c n)"),
                        scalar=lnk_g_sb[:, s:s + 1],
                        in1=posT_s[:, half * 1024:(half + 1) * 1024],
                        op0=ALU.mult, op1=ALU.add,
                    )

                for h in (2 * s, 2 * s + 1):
                    p0 = DH * (h % 2)
                    po = ps_o.tile([DH + 1, S], f32, tag="po")
                    for mc0, w in ((0, 3), (3, 3), (6, 3), (9, 3), (12, 3), (15, 1)):
                        ps = ps_s.tile([P, 3, S], f32, tag="ps3")
                        for j in range(w):
                            mc = mc0 + j
                            nc.tensor.matmul(
                                ps[:, j],
                                kT_sb[p0:p0 + DH, s, mc * P:(mc + 1) * P],
                                qT_sb[p0:p0 + DH, s, :],
                                start=True, stop=True,
                            )
                        at = apool.tile([P, 3, S], bf16, tag="at")
                        nc.scalar.activation(
                            out=at[:, :w], in_=ps[:, :w], func=ACTF.Exp, scale=SCALE
                        )
                        for j in range(w):
                            mc = mc0 + j
                            nc.tensor.matmul(
                                po, vaug_sb[:, mc, h, :], at[:, j],
                                start=(mc == 0), stop=(mc == MCH - 1),
                            )
                    rinv = hwork.tile([1, S], f32, tag="rinv")
                    nc.vector.reciprocal(out=rinv, in_=po[DH:DH + 1, :])
                    rbc = hwork.tile([DH, S], f32, tag="rbc")
                    nc.gpsimd.partition_broadcast(rbc, rinv)
                    nc.vector.tensor_tensor(
                        out=outT_sb[p0:p0 + DH, s, :], in0=po[0:DH, :],
                        in1=rbc, op=ALU.mult,
                    )

        # ========== phase Z: out-proj, gate, fuse, final LN ==========
        with (
            tc.tile_pool(name="zw", bufs=1) as zw,
            tc.tile_pool(name="zs", bufs=2) as zs,
            tc.tile_pool(name="stat3", bufs=4) as stat3,
            tc.tile_pool(name="ps_z", bufs=2, space="PSUM") as ps_z,
            tc.tile_pool(name="ps_g", bufs=2, space="PSUM") as ps_g,
        ):
            woT_sb = zw.tile([P, G6, D], bf16)
            nc.sync.dma_start(out=woT_sb, in_=woT.rearrange("(s p) o -> p s o", p=P))
            gwvT_sb = zw.tile([P, G6, D], bf16)
            nc.scalar.dma_start(
                out=gwvT_sb, in_=gwvT.rearrange("(s p) o -> p s o", p=P)
            )
            gwzT_sb = zw.tile([P, G6, D], bf16)
            nc.gpsimd.dma_start(
                out=gwzT_sb, in_=gwzT.rearrange("(s p) o -> p s o", p=P)
            )
            bo_sb = gb_sb = None
            if has_o_bias:
                bo_sb = zw.tile([1, D], bf16)
                nc.sync.dma_start(out=bo_sb, in_=bo_a.ap())
            if has_g_bias:
                gb_sb = zw.tile([1, D], bf16)
                nc.sync.dma_start(out=gb_sb, in_=gb_e.ap())
            gbc = bbc = None
            if has_f_affine:
                gbc = zw.tile([P, D], f32)
                bbc = zw.tile([P, D], f32)
                for dst, src_row in ((gbc, lnfw_sb), (bbc, lnfb_sb)):
                    pb = ps_z.tile([P, D], f32, tag="pz")
                    for o0, o1 in HALVES:
                        nc.tensor.matmul(
                            pb[:, o0:o1], ones_r_f, src_row[:, o0:o1],
                            start=True, stop=True,
                        )
                    nc.vector.tensor_copy(out=dst, in_=pb)

            for c in range(NCH):
                vis_c = zs.tile([P, D], f32, tag="vis")
                nc.sync.dma_start(
                    out=vis_c, in_=vis_nat.rearrange("(c p) o -> p c o", p=P)[:, c]
                )

                # gate: vis-half (own xv columns are [0, 512)) + folded z-half
                pg = ps_g.tile([P, D], f32, tag="pg")
                for o0, o1 in HALVES:
                    for s in range(G6):
                        nc.tensor.matmul(
                            pg[:, o0:o1],
                            xvT_sb[:, s, c * P:(c + 1) * P],
                            gwvT_sb[:, s, o0:o1],
                            start=(s == 0), stop=False,
                        )
                    for s in range(G6):
                        last = (not has_g_bias) and s == G6 - 1
                        nc.tensor.matmul(
                            pg[:, o0:o1],
                            outT_sb[:, s, c * P:(c + 1) * P],
                            gwzT_sb[:, s, o0:o1],
                            start=False, stop=last,
                        )
                    if has_g_bias:
                        nc.tensor.matmul(
                            pg[:, o0:o1], ones_b, gb_sb[:, o0:o1],
                            start=False, stop=True,
                        )
                gsig = zs.tile([P, D], bf16, tag="gsig")
                nc.scalar.activation(out=gsig, in_=pg, func=ACTF.Sigmoid)

                # out-proj z
                pz = ps_z.tile([P, D], f32, tag="pz")
                for o0, o1 in HALVES:
                    for s in range(G6):
                        last = (not has_o_bias) and s == G6 - 1
                        nc.tensor.matmul(
                            pz[:, o0:o1],
                            outT_sb[:, s, c * P:(c + 1) * P],
                            woT_sb[:, s, o0:o1],
                            start=(s == 0), stop=last,
                        )
                    if has_o_bias:
                        nc.tensor.matmul(
                            pz[:, o0:o1], ones_b, bo_sb[:, o0:o1],
                            start=False, stop=True,
                        )
                z_c = zs.tile([P, D], f32, tag="zc")
                nc.scalar.copy(out=z_c, in_=pz)

                # fuse: z + g*(vis - z)
                dvz = zs.tile([P, D], f32, tag="dvz")
                nc.gpsimd.tensor_tensor(out=dvz, in0=vis_c, in1=z_c, op=ALU.subtract)
                fus = zs.tile([P, D], f32, tag="fus")
                nc.vector.tensor_tensor(out=fus, in0=gsig, in1=dvz, op=ALU.mult)
                nc.vector.tensor_tensor(out=fus, in0=fus, in1=z_c, op=ALU.add)
                negmr, rstd = ln_stats(fus, stat3)
                tnorm = zs.tile([P, D], f32, tag="tnorm")
                nc.scalar.activation(
                    out=tnorm, in_=fus, func=ACTF.Identity, bias=negmr, scale=rstd
                )
                if has_f_affine:
                    nc.vector.tensor_tensor(out=tnorm, in0=tnorm, in1=gbc, op=ALU.mult)
                    nc.vector.tensor_tensor(out=tnorm, in0=tnorm, in1=bbc, op=ALU.add)
                nc.sync.dma_start(
                    out=out_rows.rearrange("(c p) o -> p c o", p=P)[:, c], in_=tnorm
                )

    nc.compile()
    return nc


def _prepare_in_maps(inputs):
    f32 = np.float32
    vis = np.asarray(inputs["visible_features"], f32)
    inf = np.asarray(inputs["infrared_features"], f32)
    wq = np.asarray(inputs["wq"], f32)
    bq = np.asarray(inputs["bq"], f32)
    lnq_w = np.asarray(inputs["lnq_w"], f32)
    lnq_b = np.asarray(inputs["lnq_b"], f32)
    wk = np.asarray(inputs["wk"], f32)
    bk = np.asarray(inputs["bk"], f32)
    lnk_w = np.asarray(inputs["lnk_w"], f32)
    lnk_b = np.asarray(inputs["lnk_b"], f32)
    wv = np.asarray(inputs["wv"], f32)
    bv = np.asarray(inputs["bv"], f32)
    lnv_w = np.asarray(inputs["lnv_w"], f32)
    lnv_b = np.asarray(inputs["lnv_b"], f32)
    pos = np.asarray(inputs["pos_emb"], f32)[:N]
    wo = np.asarray(inputs["wo"], f32)
    bo = np.asarray(inputs["bo"], f32)
    gw = np.asarray(inputs["gate_w"], f32)
    gb_ = np.asarray(inputs["gate_b"], f32)
    ln_w = np.asarray(inputs["ln_w"], f32)
    ln_b = np.asarray(inputs["ln_b"], f32)

    wo_eff = wo * lnv_w[None, :]
    bo_a = bo + wo @ lnv_b
    gwv = gw[:, :D]
    gwz = gw[:, D:]
    W_eff = (gwz.astype(np.float64) @ wo_eff.astype(np.float64)).astype(f32)
    gb_eff = gb_ + gwz @ bo_a

    wqT = np.ascontiguousarray(wq.T)
    wkvT = np.ascontiguousarray(np.concatenate([wk.T, wv.T], axis=1)).astype(BF)
    woTb = np.ascontiguousarray(wo_eff.T).astype(BF)
    gwvTb = np.ascontiguousarray(gwv.T).astype(BF)
    gwzTb = np.ascontiguousarray(W_eff.T).astype(BF)
    bqkv = np.ascontiguousarray(np.concatenate([bq, bk, bv])[None])
    bo_ab = np.ascontiguousarray(bo_a[None]).astype(BF)
    gb_eb = np.ascontiguousarray(gb_eff[None]).astype(BF)
    lnq_g2 = np.ascontiguousarray(lnq_w.reshape(G6, P).T)
    lnq_b2 = np.ascontiguousarray(lnq_b.reshape(G6, P).T)
    lnk_g2 = np.ascontiguousarray(lnk_w.reshape(G6, P).T)
    lnf = np.stack([ln_w, ln_b])

    flags = (
        bool(np.any(bq) or np.any(bk) or np.any(bv)),
        bool(np.any(bo_a)),
        bool(np.any(gb_eff)),
        bool(np.any(ln_b) or np.any(ln_w != 1.0)),
    )

    posT_base = pos.T / SCALE + lnk_b[:, None]   # [D, N]

    in_maps = []
    for c in range(CORES):
        b, r0 = c // GROUP, (c % GROUP) * S
        perm = np.concatenate(
            [np.arange(r0, r0 + S), np.arange(0, r0), np.arange(r0 + S, N)]
        )
        in_maps.append({
            "xqT": np.ascontiguousarray(inf[b, r0:r0 + S].T),
            "xvT": np.ascontiguousarray(vis[b][perm].T).astype(BF),
            "vis_nat": np.ascontiguousarray(vis[b, r0:r0 + S]),
            "posTb": np.ascontiguousarray(posT_base[:, perm]).astype(BF),
            "wqT": wqT,
            "wkvT": wkvT,
            "woT": woTb,
            "gwvT": gwvTb,
            "gwzT": gwzTb,
            "lnq_g": lnq_g2,
            "lnq_b": lnq_b2,
            "lnk_g": lnk_g2,
            "bqkv": bqkv,
            "bo_a": bo_ab,
            "gb_e": gb_eb,
            "lnf": lnf,
        })
    return in_maps, flags


def kernel(trace=False, **inputs):
    from concourse.bass_utils import run_bass_kernel_spmd

    in_maps, flags = _prepare_in_maps(inputs)
    key = ("nc",) + flags
    if key not in _CACHE:
        _CACHE[key] = _build(*flags)
    nc = _CACHE[key]
    res = run_bass_kernel_spmd(
        nc, in_maps, core_ids=list(range(CORES)), trace=trace
    )
    out = np.empty((B, N, D), np.float32)
    for c in range(CORES):
        b, r0 = c // GROUP, (c % GROUP) * S
        out[b, r0:r0 + S] = res.results[c]["out_rows"]
    _CACHE["last_result"] = res
    _CACHE["nc"] = nc
    return out


# revision 16
# speedup vs baseline: 1.8222x; 1.0490x over previous
"""Trainium2 Bass kernel for nn_CustomCrossModalAttention (B=2, N=2048, D=768, H=12).

Sharding (8 cores, collective-free):
  - core c owns batch b = c//4 and query rows [512*(c%4), 512*(c%4)+512).
  - k'/v are computed REDUNDANTLY for the whole batch on each of its 4 cores
    (~45us extra PE) instead of exchanging shards: the AllGather pair cost far
    more than the replicated matmuls and serialized the whole pipeline.
  - Keys are column-PERMUTED per core so the core's own 512 rows come first
    (softmax sums over all keys, so key order is irrelevant); this makes the
    SPMD program uniform while the gate still reads "own" xv columns at a
    fixed offset 0.

Math folds (exact):
  - scores = (q@k^T)*scale + q@pos^T == scale * (q @ (LNk*g + lnk_b + pos/scale)^T)
  - LN_v gain/bias folded into wo/bo.
  - gate z-half folded through the output projection: gate = sigmoid(
      vis@gwv^T + attnout@(gwz@wo_eff)^T + gb + gwz@bo_a), removing the
    z -> zT transposes and the serialization on z.
  - All additive biases in this problem are structurally zero
    (setup_inputs uses jnp.zeros); nonzero biases are supported via
    ones-row matmuls compiled on demand (flags in the build cache key).

Dtypes (validated by numpy emulation to rel-err ~8e-3, same as the old
AllGather kernel): q path f32r end-to-end (q errors multiply the large q@pos
term in the exp argument, so bf16 there would cost ~2% at-error); k/v/gate/out
paths bf16; kT kept f32 (magnitude ~8 after the pos fold); exp/softmax in
bf16; all matmul accumulation f32 in PSUM.

Schedule: V-proj -> Q-proj -> K-proj -> per-head-pair [kT transpose block ->
attention heads 2s,2s+1] -> out-proj/gate/fuse/final-LN. Interleaving the kT
blocks with attention lets the Activation engine's softmax exp (~95us, the
2nd-busiest engine) start while PE is still projecting.
"""

import numpy as np
import ml_dtypes

B, N, D = 2, 2048, 768
H, DH = 12, 64
P = 128
CORES, GROUP = 8, 4
S = 512            # query rows per core
NCH = S // P       # 4 row chunks per core
MCH = N // P       # 16 key chunks
G6 = D // P        # 6
SCALE = DH ** -0.5
EPS = 1e-5

BF = ml_dtypes.bfloat16

_CACHE = {}

HALVES = [(0, 512), (512, D)]


def _build(has_qkv_bias, has_o_bias, has_g_bias, has_f_affine):
    from contextlib import ExitStack

    import concourse.bacc as bacc
    import concourse.mybir as mybir
    import concourse.tile as tile
    from concourse.masks import make_identity

    f32 = mybir.dt.float32
    f32r = mybir.dt.float32r
    bf16 = mybir.dt.bfloat16
    ALU = mybir.AluOpType
    ACTF = mybir.ActivationFunctionType

    nc = bacc.Bacc("TRN2", target_bir_lowering=False, num_devices=CORES)

    def din(name, shape, dt=bf16):
        return nc.dram_tensor(name, shape, dt, kind="ExternalInput")

    xqT = din("xqT", [D, S], f32r)        # own infrared rows, transposed
    xvT = din("xvT", [D, N], bf16)        # full-batch visible, transposed, key-permuted
    vis_nat = din("vis_nat", [S, D], f32)  # own visible rows, natural
    posTb = din("posTb", [D, N], bf16)    # pos/scale + lnk_b, transposed, permuted
    wqT = din("wqT", [D, D], f32r)
    wkvT = din("wkvT", [D, 2 * D], bf16)  # [wk.T | wv.T]
    woT = din("woT", [D, D], bf16)        # (wo * lnv_w).T
    gwvT = din("gwvT", [D, D], bf16)      # gate vis-half weights, transposed
    gwzT = din("gwzT", [D, D], bf16)      # (gwz @ wo_eff).T
    lnq_g = din("lnq_g", [P, G6], f32)
    lnq_b = din("lnq_b", [P, G6], f32)
    lnk_g = din("lnk_g", [P, G6], f32)
    bqkv = din("bqkv", [1, 3 * D], f32r)  # only read when has_qkv_bias
    bo_a = din("bo_a", [1, D], bf16)      # bo + wo@lnv_b
    gb_e = din("gb_e", [1, D], bf16)      # gate_b + gwz@bo_a
    lnf = din("lnf", [2, D], f32)
    out_rows = nc.dram_tensor("out_rows", [S, D], f32, kind="ExternalOutput")

    with tile.TileContext(nc) as tc, ExitStack() as ctx:
        const = ctx.enter_context(tc.tile_pool(name="const", bufs=1))
        persist = ctx.enter_context(tc.tile_pool(name="persist", bufs=1))

        ident_bf = const.tile([P, P], bf16)
        make_identity(nc, ident_bf)
        ident_f32 = const.tile([P, P], f32)
        make_identity(nc, ident_f32)
        ones_r_f = const.tile([1, P], f32)
        nc.vector.memset(ones_r_f, 1.0)
        ones_r = ones_r_f.bitcast(f32r)
        ones_b = const.tile([1, P], bf16)
        nc.vector.memset(ones_b, 1.0)
        eps_t = const.tile([P, 1], f32)
        nc.vector.memset(eps_t, EPS)

        # ---- persistent tiles ----
        xvT_sb = persist.tile([P, G6, N], bf16)
        kT_sb = persist.tile([P, G6, N], f32r)
        vaug_sb = persist.tile([P, MCH, H, DH + 1], bf16)
        qT_sb = persist.tile([P, G6, S], f32r)
        outT_sb = persist.tile([P, G6, S], bf16)
        lnq_g_sb = persist.tile([P, G6], f32)
        lnq_b_sb = persist.tile([P, G6], f32)
        lnk_g_sb = persist.tile([P, G6], f32)
        lnfw_sb = persist.tile([1, D], f32)
        lnfb_sb = persist.tile([1, D], f32)

        nc.vector.memset(vaug_sb[:, :, :, DH:DH + 1], 1.0)
        nc.gpsimd.dma_start(out=lnq_g_sb, in_=lnq_g.ap())
        nc.gpsimd.dma_start(out=lnq_b_sb, in_=lnq_b.ap())
        nc.gpsimd.dma_start(out=lnk_g_sb, in_=lnk_g.ap())
        nc.gpsimd.dma_start(out=lnfw_sb, in_=lnf.ap()[0:1, :])
        nc.gpsimd.dma_start(out=lnfb_sb, in_=lnf.ap()[1:2, :])

        def ln_stats(y, pool):
            st = pool.tile([P, 2, 6], f32, tag="st")
            for i in range(2):
                nc.vector.bn_stats(out=st[:, i], in_=y[:, i * 384:(i + 1) * 384])
            mv = pool.tile([P, 2], f32, tag="mv")
            nc.vector.bn_aggr(out=mv, in_=st)
            rstd = pool.tile([P, 1], f32, tag="rstd")
            nc.scalar.activation(
                out=rstd, in_=mv[:, 1:2], func=ACTF.Sqrt, bias=eps_t, scale=1.0
            )
            nc.vector.reciprocal(out=rstd, in_=rstd)
            negmr = pool.tile([P, 1], f32, tag="negmr")
            nc.vector.tensor_scalar(
                out=negmr, in0=mv[:, 0:1], scalar1=rstd, scalar2=-1.0,
                op0=ALU.mult, op1=ALU.mult,
            )
            return negmr, rstd

        # ================= phase V + Q (scoped) =================
        with (
            tc.tile_pool(name="pvq", bufs=1) as pvq,
            tc.tile_pool(name="wrot", bufs=2) as wrot,
            tc.tile_pool(name="stat", bufs=6) as stat,
            tc.tile_pool(name="qn", bufs=1) as qn,
            tc.tile_pool(name="ps_p", bufs=3, space="PSUM") as ps_p,
            tc.tile_pool(name="ps_t", bufs=2, space="PSUM") as ps_t,
        ):
            # DMA issue order == need order: wv, xvT chunks, then q weights
            wv_sb = wrot.tile([P, G6, D], bf16, tag="wkv")
            nc.sync.dma_start(
                out=wv_sb,
                in_=wkvT.rearrange("(s p) o -> p s o", p=P)[:, :, D:2 * D],
            )
            for i in range(4):
                nc.gpsimd.dma_start(
                    out=xvT_sb[:, :, i * S:(i + 1) * S],
                    in_=xvT.rearrange("(s p) n -> p s n", p=P)[:, :, i * S:(i + 1) * S],
                )
            bqkv_sb = None
            if has_qkv_bias:
                bqkv_sb = pvq.tile([1, 3 * D], f32r)
                nc.sync.dma_start(out=bqkv_sb, in_=bqkv.ap())

            def proj_tile(lhsT_sb, w_sb, w_off, c):
                py = ps_p.tile([P, D], f32, tag="py")
                for o0, o1 in HALVES:
                    for s in range(G6):
                        nc.tensor.matmul(
                            py[:, o0:o1],
                            lhsT_sb[:, s, c * P:(c + 1) * P],
                            w_sb[:, s, o0:o1],
                            start=(s == 0), stop=(not has_qkv_bias and s == G6 - 1),
                        )
                    if has_qkv_bias:
                        nc.tensor.matmul(
                            py[:, o0:o1], ones_r,
                            bqkv_sb[:, w_off + o0:w_off + o1],
                            start=False, stop=True,
                        )
                return py

            xqT_sb = pvq.tile([P, G6, S], f32r)
            nc.scalar.dma_start(
                out=xqT_sb, in_=xqT.rearrange("(s p) n -> p s n", p=P)
            )
            wq_sb = pvq.tile([P, G6, D], f32r)
            nc.sync.dma_start(
                out=wq_sb, in_=wqT.rearrange("(s p) o -> p s o", p=P)
            )

            # ---- V: 16 chunks, straight into vaug ----
            for c in range(MCH):
                py = proj_tile(xvT_sb, wv_sb, 2 * D, c)
                negmr, rstd = ln_stats(py, stat)
                nc.scalar.activation(
                    out=vaug_sb[:, c, :, 0:DH],
                    in_=py.rearrange("p (h d) -> p h d", h=H),
                    func=ACTF.Identity, bias=negmr, scale=rstd,
                )

            # ---- Q: 4 chunks ----
            qnats = []
            for c in range(NCH):
                py = proj_tile(xqT_sb, wq_sb, 0, c)
                negmr, rstd = ln_stats(py, stat)
                qnat = qn.tile([P, D], f32, tag=f"qnat{c}")
                nc.scalar.activation(
                    out=qnat, in_=py, func=ACTF.Identity, bias=negmr, scale=rstd
                )
                qnats.append(qnat)
            for s in range(G6):
                pt = ps_t.tile([P, NCH, P], f32, tag="pt")
                for c in range(NCH):
                    nc.tensor.transpose(
                        pt[:, c], qnats[c][:, s * P:(s + 1) * P], ident_f32
                    )
                nc.vector.tensor_scalar(
                    out=qT_sb[:, s, :],
                    in0=pt.rearrange("p c n -> p (c n)"),
                    scalar1=lnq_g_sb[:, s:s + 1],
                    scalar2=lnq_b_sb[:, s:s + 1],
                    op0=ALU.mult, op1=ALU.add,
                )

        # ============ phase K + attention (interleaved) ============
        with (
            tc.tile_pool(name="kp", bufs=1) as kp,
            tc.tile_pool(name="wrot2", bufs=1) as wrot2,
            tc.tile_pool(name="post", bufs=2) as postp,
            tc.tile_pool(name="stat2", bufs=6) as stat2,
            tc.tile_pool(name="attn", bufs=3) as apool,
            tc.tile_pool(name="hwork", bufs=4) as hwork,
        ):
            wk_sb = wrot2.tile([P, G6, D], bf16)
            nc.sync.dma_start(
                out=wk_sb,
                in_=wkvT.rearrange("(s p) o -> p s o", p=P)[:, :, 0:D],
            )
            knat_sb = kp.tile([P, MCH, D], bf16)

            with tc.tile_pool(name="ps_pk", bufs=3, space="PSUM") as ps_pk:
                for c in range(MCH):
                    py = ps_pk.tile([P, D], f32, tag="pyk")
                    for o0, o1 in HALVES:
                        for s in range(G6):
                            nc.tensor.matmul(
                                py[:, o0:o1],
                                xvT_sb[:, s, c * P:(c + 1) * P],
                                wk_sb[:, s, o0:o1],
                                start=(s == 0),
                                stop=(not has_qkv_bias and s == G6 - 1),
                            )
                        if has_qkv_bias:
                            nc.tensor.matmul(
                                py[:, o0:o1], ones_r, bqkv_sb[:, D + o0:D + o1],
                                start=False, stop=True,
                            )
                    negmr, rstd = ln_stats(py, stat2)
                    nc.scalar.activation(
                        out=knat_sb[:, c, :], in_=py, func=ACTF.Identity,
                        bias=negmr, scale=rstd,
                    )

            attn_psum = ExitStack()
            ps_s = attn_psum.enter_context(
                tc.tile_pool(name="ps_s", bufs=2, space="PSUM")
            )
            ps_o = attn_psum.enter_context(
                tc.tile_pool(name="ps_o", bufs=2, space="PSUM")
            )
            # per head-pair s: build kT block s, then run heads 2s, 2s+1
            for s in range(G6):
                posT_s = postp.tile([P, N], bf16, tag="posT")
                nc.scalar.dma_start(
                    out=posT_s,
                    in_=posTb.rearrange("(s p) n -> p s n", p=P)[:, s, :],
                )
                for half in range(2):
                    pt = ps_s.tile([P, MCH // 2, P], bf16, tag="ps3")
                    for c in range(MCH // 2):
                        mc = half * 8 + c
                        nc.tensor.transpose(
                            pt[:, c], knat_sb[:, mc, s * P:(s + 1) * P], ident_bf
                        )
                    nc.vector.scalar_tensor_tensor(
                        out=kT_sb[:, s, half * 1024:(half + 1) * 1024],
                        in0=pt.rearrange("p c n -> p (c n)"),
                        scalar=lnk_g_sb[:, s:s + 1],
                        in1=posT_s[:, half * 1024:(half + 1) * 1024],
                        op0=ALU.mult, op1=ALU.add,
                    )

                for h in (2 * s, 2 * s + 1):
                    p0 = DH * (h % 2)
                    po = ps_o.tile([DH + 1, S], f32, tag="po")
                    for mc0, w in ((0, 3), (3, 3), (6, 3), (9, 3), (12, 3), (15, 1)):
                        ps = ps_s.tile([P, 3, S], f32, tag="ps3")
                        for j in range(w):
                            mc = mc0 + j
                            nc.tensor.matmul(
                                ps[:, j],
                                kT_sb[p0:p0 + DH, s, mc * P:(mc + 1) * P],
                                qT_sb[p0:p0 + DH, s, :],
                                start=True, stop=True,
                            )
                        at = apool.tile([P, 3, S], bf16, tag="at")
                        nc.scalar.activation(
                            out=at[:, :w], in_=ps[:, :w], func=ACTF.Exp, scale=SCALE
                        )
                        for j in range(w):
                            mc = mc0 + j
                            nc.tensor.matmul(
                                po, vaug_sb[:, mc, h, :], at[:, j],
                                start=(mc == 0), stop=(mc == MCH - 1),
                            )
                    rinv = hwork.tile([1, S], f32, tag="rinv")
                    nc.vector.reciprocal(out=rinv, in_=po[DH:DH + 1, :])
                    rbc = hwork.tile([DH, S], f32, tag="rbc")
                    nc.gpsimd.partition_broadcast(rbc, rinv)
                    nc.vector.tensor_tensor(
                        out=outT_sb[p0:p0 + DH, s, :], in0=po[0:DH, :],
                        in1=rbc, op=ALU.mult,
                    )
            attn_psum.close()

        # ========== phase Z: out-proj, gate, fuse, final LN ==========
        with (
            tc.tile_pool(name="zw", bufs=1) as zw,
            tc.tile_pool(name="zs", bufs=2) as zs,
            tc.tile_pool(name="stat3", bufs=4) as stat3,
            tc.tile_pool(name="ps_z", bufs=2, space="PSUM") as ps_z,
            tc.tile_pool(name="ps_g", bufs=2, space="PSUM") as ps_g,
        ):
            woT_sb = zw.tile([P, G6, D], bf16)
            nc.sync.dma_start(out=woT_sb, in_=woT.rearrange("(s p) o -> p s o", p=P))
            gwvT_sb = zw.tile([P, G6, D], bf16)
            nc.scalar.dma_start(
                out=gwvT_sb, in_=gwvT.rearrange("(s p) o -> p s o", p=P)
            )
            gwzT_sb = zw.tile([P, G6, D], bf16)
            nc.gpsimd.dma_start(
                out=gwzT_sb, in_=gwzT.rearrange("(s p) o -> p s o", p=P)
            )
            bo_sb = gb_sb = None
            if has_o_bias:
                bo_sb = zw.tile([1, D], bf16)
                nc.sync.dma_start(out=bo_sb, in_=bo_a.ap())
            if has_g_bias:
                gb_sb = zw.tile([1, D], bf16)
                nc.sync.dma_start(out=gb_sb, in_=gb_e.ap())
            gbc = bbc = None
            if has_f_affine:
                gbc = zw.tile([P, D], f32)
                bbc = zw.tile([P, D], f32)
                for dst, src_row in ((gbc, lnfw_sb), (bbc, lnfb_sb)):
                    pb = ps_z.tile([P, D], f32, tag="pz")
                    for o0, o1 in HALVES:
                        nc.tensor.matmul(
                            pb[:, o0:o1], ones_r_f, src_row[:, o0:o1],
                            start=True, stop=True,
                        )
                    nc.vector.tensor_copy(out=dst, in_=pb)

            for c in range(NCH):
                vis_c = zs.tile([P, D], f32, tag="vis")
                nc.sync.dma_start(
                    out=vis_c, in_=vis_nat.rearrange("(c p) o -> p c o", p=P)[:, c]
                )

                # gate: vis-half (own xv columns are [0, 512)) + folded z-half
                pg = ps_g.tile([P, D], f32, tag="pg")
                for o0, o1 in HALVES:
                    for s in range(G6):
                        nc.tensor.matmul(
                            pg[:, o0:o1],
                            xvT_sb[:, s, c * P:(c + 1) * P],
                            gwvT_sb[:, s, o0:o1],
                            start=(s == 0), stop=False,
                        )
                    for s in range(G6):
                        last = (not has_g_bias) and s == G6 - 1
                        nc.tensor.matmul(
                            pg[:, o0:o1],
                            outT_sb[:, s, c * P:(c + 1) * P],
                            gwzT_sb[:, s, o0:o1],
                            start=False, stop=last,
                        )
                    if has_g_bias:
                        nc.tensor.matmul(
                            pg[:, o0:o1], ones_b, gb_sb[:, o0:o1],
                            start=False, stop=True,
                        )
                gsig = zs.tile([P, D], bf16, tag="gsig")
                nc.scalar.activation(out=gsig, in_=pg, func=ACTF.Sigmoid)

                # out-proj z
                pz = ps_z.tile([P, D], f32, tag="pz")
                for o0, o1 in HALVES:
                    for s in range(G6):
                        last = (not has_o_bias) and s == G6 - 1
                        nc.tensor.matmul(
                            pz[:, o0:o1],
                            outT_sb[:, s, c * P:(c + 1) * P],
                            woT_sb[:, s, o0:o1],
                            start=(s == 0), stop=last,
                        )
                    if has_o_bias:
                        nc.tensor.matmul(
                            pz[:, o0:o1], ones_b, bo_sb[:, o0:o1],
                            start=False, stop=True,
                        )
                z_c = zs.tile([P, D], f32, tag="zc")
                nc.scalar.copy(out=z_c, in_=pz)

                # fuse: z + g*(vis - z)
                dvz = zs.tile([P, D], f32, tag="dvz")
                nc.gpsimd.tensor_tensor(out=dvz, in0=vis_c, in1=z_c, op=ALU.subtract)
                fus = zs.tile([P, D], f32, tag="fus")
                nc.vector.tensor_tensor(out=fus, in0=gsig, in1=dvz, op=ALU.mult)
                nc.vector.tensor_tensor(out=fus, in0=fus, in1=z_c, op=ALU.add)
                negmr, rstd = ln_stats(fus, stat3)
                tnorm = zs.tile([P, D], f32, tag="tnorm")
                nc.scalar.activation(
                    out=tnorm, in_=fus, func=ACTF.Identity, bias=negmr, scale=rstd
                )
                if has_f_affine:
                    nc.vector.tensor_tensor(out=tnorm, in0=tnorm, in1=gbc, op=ALU.mult)
                    nc.vector.tensor_tensor(out=tnorm, in0=tnorm, in1=bbc, op=ALU.add)
                nc.sync.dma_start(
                    out=out_rows.rearrange("(c p) o -> p c o", p=P)[:, c], in_=tnorm
                )

    nc.compile()
    return nc


def _prepare_in_maps(inputs):
    f32 = np.float32
    vis = np.asarray(inputs["visible_features"], f32)
    inf = np.asarray(inputs["infrared_features"], f32)
    wq = np.asarray(inputs["wq"], f32)
    bq = np.asarray(inputs["bq"], f32)
    lnq_w = np.asarray(inputs["lnq_w"], f32)
    lnq_b = np.asarray(inputs["lnq_b"], f32)
    wk = np.asarray(inputs["wk"], f32)
    bk = np.asarray(inputs["bk"], f32)
    lnk_w = np.asarray(inputs["lnk_w"], f32)
    lnk_b = np.asarray(inputs["lnk_b"], f32)
    wv = np.asarray(inputs["wv"], f32)
    bv = np.asarray(inputs["bv"], f32)
    lnv_w = np.asarray(inputs["lnv_w"], f32)
    lnv_b = np.asarray(inputs["lnv_b"], f32)
    pos = np.asarray(inputs["pos_emb"], f32)[:N]
    wo = np.asarray(inputs["wo"], f32)
    bo = np.asarray(inputs["bo"], f32)
    gw = np.asarray(inputs["gate_w"], f32)
    gb_ = np.asarray(inputs["gate_b"], f32)
    ln_w = np.asarray(inputs["ln_w"], f32)
    ln_b = np.asarray(inputs["ln_b"], f32)

    wo_eff = wo * lnv_w[None, :]
    bo_a = bo + wo @ lnv_b
    gwv = gw[:, :D]
    gwz = gw[:, D:]
    W_eff = (gwz.astype(np.float64) @ wo_eff.astype(np.float64)).astype(f32)
    gb_eff = gb_ + gwz @ bo_a

    wqT = np.ascontiguousarray(wq.T)
    wkvT = np.ascontiguousarray(np.concatenate([wk.T, wv.T], axis=1)).astype(BF)
    woTb = np.ascontiguousarray(wo_eff.T).astype(BF)
    gwvTb = np.ascontiguousarray(gwv.T).astype(BF)
    gwzTb = np.ascontiguousarray(W_eff.T).astype(BF)
    bqkv = np.ascontiguousarray(np.concatenate([bq, bk, bv])[None])
    bo_ab = np.ascontiguousarray(bo_a[None]).astype(BF)
    gb_eb = np.ascontiguousarray(gb_eff[None]).astype(BF)
    lnq_g2 = np.ascontiguousarray(lnq_w.reshape(G6, P).T)
    lnq_b2 = np.ascontiguousarray(lnq_b.reshape(G6, P).T)
    lnk_g2 = np.ascontiguousarray(lnk_w.reshape(G6, P).T)
    lnf = np.stack([ln_w, ln_b])

    flags = (
        bool(np.any(bq) or np.any(bk) or np.any(bv)),
        bool(np.any(bo_a)),
        bool(np.any(gb_eff)),
        bool(np.any(ln_b) or np.any(ln_w != 1.0)),
    )

    posT_base = pos.T / SCALE + lnk_b[:, None]   # [D, N]

    in_maps = []
    for c in range(CORES):
        b, r0 = c // GROUP, (c % GROUP) * S
        perm = np.concatenate(
            [np.arange(r0, r0 + S), np.arange(0, r0), np.arange(r0 + S, N)]
        )
        in_maps.append({
            "xqT": np.ascontiguousarray(inf[b, r0:r0 + S].T),
            "xvT": np.ascontiguousarray(vis[b][perm].T).astype(BF),
            "vis_nat": np.ascontiguousarray(vis[b, r0:r0 + S]),
            "posTb": np.ascontiguousarray(posT_base[:, perm]).astype(BF),
            "wqT": wqT,
            "wkvT": wkvT,
            "woT": woTb,
            "gwvT": gwvTb,
            "gwzT": gwzTb,
            "lnq_g": lnq_g2,
            "lnq_b": lnq_b2,
            "lnk_g": lnk_g2,
            "bqkv": bqkv,
            "bo_a": bo_ab,
            "gb_e": gb_eb,
            "lnf": lnf,
        })
    return in_maps, flags


def kernel(trace=False, **inputs):
    from concourse.bass_utils import run_bass_kernel_spmd

    in_maps, flags = _prepare_in_maps(inputs)
    key = ("nc",) + flags
    if key not in _CACHE:
        _CACHE[key] = _build(*flags)
    nc = _CACHE[key]
    res = run_bass_kernel_spmd(
        nc, in_maps, core_ids=list(range(CORES)), trace=trace
    )
    out = np.empty((B, N, D), np.float32)
    for c in range(CORES):
        b, r0 = c // GROUP, (c % GROUP) * S
        out[b, r0:r0 + S] = res.results[c]["out_rows"]
    _CACHE["last_result"] = res
    _CACHE["nc"] = nc
    return out


# revision 21
# speedup vs baseline: 1.9671x; 1.0795x over previous
"""Trainium2 Bass kernel for nn_CustomCrossModalAttention (B=2, N=2048, D=768, H=12).

Sharding (8 cores, collective-free):
  - core c owns batch b = c//4 and query rows [512*(c%4), 512*(c%4)+512).
  - k'/v are computed REDUNDANTLY for the whole batch on each of its 4 cores
    (~45us extra PE) instead of exchanging shards: the AllGather pair cost far
    more than the replicated matmuls and serialized the whole pipeline.
  - Keys are column-PERMUTED per core so the core's own 512 rows come first
    (softmax sums over all keys, so key order is irrelevant); this makes the
    SPMD program uniform while the gate still reads "own" xv columns at a
    fixed offset 0.

Math folds (exact):
  - scores = (q@k^T)*scale + q@pos^T == scale * (q @ (LNk*g + lnk_b + pos/scale)^T)
  - LN_v gain/bias folded into wo/bo.
  - gate z-half folded through the output projection: gate = sigmoid(
      vis@gwv^T + attnout@(gwz@wo_eff)^T + gb + gwz@bo_a), removing the
    z -> zT transposes and the serialization on z.
  - All additive biases in this problem are structurally zero
    (setup_inputs uses jnp.zeros); nonzero biases are supported via
    ones-row matmuls compiled on demand (flags in the build cache key).

Dtypes (validated by numpy emulation to rel-err ~8e-3, same as the old
AllGather kernel): q path f32r end-to-end (q errors multiply the large q@pos
term in the exp argument, so bf16 there would cost ~2% at-error); k/v/gate/out
paths bf16; kT kept f32 (magnitude ~8 after the pos fold); exp/softmax in
bf16; all matmul accumulation f32 in PSUM.

Schedule: V-proj -> Q-proj -> K-proj -> per-head-pair [kT transpose block ->
attention heads 2s,2s+1] -> out-proj/gate/fuse/final-LN. Interleaving the kT
blocks with attention lets the Activation engine's softmax exp (~95us, the
2nd-busiest engine) start while PE is still projecting.
"""

import numpy as np
import ml_dtypes

B, N, D = 2, 2048, 768
H, DH = 12, 64
P = 128
CORES, GROUP = 8, 4
S = 512            # query rows per core
NCH = S // P       # 4 row chunks per core
MCH = N // P       # 16 key chunks
G6 = D // P        # 6
SCALE = DH ** -0.5
EPS = 1e-5

BF = ml_dtypes.bfloat16

_CACHE = {}

HALVES = [(0, 512), (512, D)]


def _build(has_qkv_bias, has_o_bias, has_g_bias, has_f_affine):
    from contextlib import ExitStack

    import concourse.bacc as bacc
    import concourse.mybir as mybir
    import concourse.tile as tile
    from concourse.masks import make_identity

    f32 = mybir.dt.float32
    f32r = mybir.dt.float32r
    bf16 = mybir.dt.bfloat16
    ALU = mybir.AluOpType
    ACTF = mybir.ActivationFunctionType

    nc = bacc.Bacc("TRN2", target_bir_lowering=False, num_devices=CORES)

    def din(name, shape, dt=bf16):
        return nc.dram_tensor(name, shape, dt, kind="ExternalInput")

    xqT = din("xqT", [D, S], f32r)        # own infrared rows, transposed
    xvT = din("xvT", [D, N], bf16)        # full-batch visible, transposed, key-permuted
    vis_nat = din("vis_nat", [S, D], f32)  # own visible rows, natural
    posTb = din("posTb", [D, N], bf16)    # pos/scale + lnk_b, transposed, permuted
    wqT = din("wqT", [D, D], f32r)
    wkvT = din("wkvT", [D, 2 * D], bf16)  # [wk.T | wv.T]
    woT = din("woT", [D, D], bf16)        # (wo * lnv_w).T
    gwvT = din("gwvT", [D, D], bf16)      # gate vis-half weights, transposed
    gwzT = din("gwzT", [D, D], bf16)      # (gwz @ wo_eff).T
    lnq_g = din("lnq_g", [P, G6], f32)
    lnq_b = din("lnq_b", [P, G6], f32)
    lnk_g = din("lnk_g", [P, G6], f32)
    bqkv = din("bqkv", [1, 3 * D], f32r)  # only read when has_qkv_bias
    bo_a = din("bo_a", [1, D], bf16)      # bo + wo@lnv_b
    gb_e = din("gb_e", [1, D], bf16)      # gate_b + gwz@bo_a
    lnf = din("lnf", [2, D], f32)
    out_rows = nc.dram_tensor("out_rows", [S, D], f32, kind="ExternalOutput")

    with tile.TileContext(nc) as tc, ExitStack() as ctx:
        const = ctx.enter_context(tc.tile_pool(name="const", bufs=1))
        persist = ctx.enter_context(tc.tile_pool(name="persist", bufs=1))

        ident_bf = const.tile([P, P], bf16)
        make_identity(nc, ident_bf)
        ident_f32 = const.tile([P, P], f32)
        make_identity(nc, ident_f32)
        ones_r_f = const.tile([1, P], f32)
        nc.vector.memset(ones_r_f, 1.0)
        ones_r = ones_r_f.bitcast(f32r)
        ones_b = const.tile([1, P], bf16)
        nc.vector.memset(ones_b, 1.0)
        eps_t = const.tile([P, 1], f32)
        nc.vector.memset(eps_t, EPS)

        # ---- persistent tiles ----
        xvT_sb = persist.tile([P, G6, N], bf16)
        kT_sb = persist.tile([P, G6, N], f32r)
        vaug_sb = persist.tile([P, MCH, H, DH + 1], bf16)
        qT_sb = persist.tile([P, G6, S], f32r)
        outT_sb = persist.tile([P, G6, S], bf16)
        lnq_g_sb = persist.tile([P, G6], f32)
        lnq_b_sb = persist.tile([P, G6], f32)
        lnk_g_sb = persist.tile([P, G6], f32)
        lnfw_sb = persist.tile([1, D], f32)
        lnfb_sb = persist.tile([1, D], f32)

        nc.vector.memset(vaug_sb[:, :, :, DH:DH + 1], 1.0)

        def ln_stats(y, pool):
            st = pool.tile([P, 2, 6], f32, tag="st")
            for i in range(2):
                nc.vector.bn_stats(out=st[:, i], in_=y[:, i * 384:(i + 1) * 384])
            mv = pool.tile([P, 2], f32, tag="mv")
            nc.vector.bn_aggr(out=mv, in_=st)
            rstd = pool.tile([P, 1], f32, tag="rstd")
            nc.scalar.activation(
                out=rstd, in_=mv[:, 1:2], func=ACTF.Sqrt, bias=eps_t, scale=1.0
            )
            nc.vector.reciprocal(out=rstd, in_=rstd)
            negmr = pool.tile([P, 1], f32, tag="negmr")
            nc.vector.tensor_scalar(
                out=negmr, in0=mv[:, 0:1], scalar1=rstd, scalar2=-1.0,
                op0=ALU.mult, op1=ALU.mult,
            )
            return negmr, rstd

        # ================= phase V + Q (scoped) =================
        with (
            tc.tile_pool(name="pvq", bufs=1) as pvq,
            tc.tile_pool(name="wrot", bufs=2) as wrot,
            tc.tile_pool(name="stat", bufs=6) as stat,
            tc.tile_pool(name="qn", bufs=1) as qn,
            tc.tile_pool(name="ps_p", bufs=3, space="PSUM") as ps_p,
            tc.tile_pool(name="ps_t", bufs=2, space="PSUM") as ps_t,
        ):
            # DMA issue order == need order: xvT chunk 0 and wv first (V-proj
            # starts on them), remaining xvT chunks, then the q-path tensors.
            nc.sync.dma_start(
                out=xvT_sb[:, :, 0:S],
                in_=xvT.rearrange("(s p) n -> p s n", p=P)[:, :, 0:S],
            )
            wv_sb = wrot.tile([P, G6, D], bf16, tag="wkv")
            nc.scalar.dma_start(
                out=wv_sb,
                in_=wkvT.rearrange("(s p) o -> p s o", p=P)[:, :, D:2 * D],
            )
            for i in range(1, 4):
                nc.gpsimd.dma_start(
                    out=xvT_sb[:, :, i * S:(i + 1) * S],
                    in_=xvT.rearrange("(s p) n -> p s n", p=P)[:, :, i * S:(i + 1) * S],
                )
            bqkv_sb = None
            if has_qkv_bias:
                bqkv_sb = pvq.tile([1, 3 * D], f32r)
                nc.sync.dma_start(out=bqkv_sb, in_=bqkv.ap())

            def proj_tile(lhsT_sb, w_sb, w_off, c):
                py = ps_p.tile([P, D], f32, tag="py")
                for o0, o1 in HALVES:
                    for s in range(G6):
                        nc.tensor.matmul(
                            py[:, o0:o1],
                            lhsT_sb[:, s, c * P:(c + 1) * P],
                            w_sb[:, s, o0:o1],
                            start=(s == 0), stop=(not has_qkv_bias and s == G6 - 1),
                        )
                    if has_qkv_bias:
                        nc.tensor.matmul(
                            py[:, o0:o1], ones_r,
                            bqkv_sb[:, w_off + o0:w_off + o1],
                            start=False, stop=True,
                        )
                return py

            xqT_sb = pvq.tile([P, G6, S], f32r)
            nc.scalar.dma_start(
                out=xqT_sb, in_=xqT.rearrange("(s p) n -> p s n", p=P)
            )
            wq_sb = pvq.tile([P, G6, D], f32r)
            nc.sync.dma_start(
                out=wq_sb, in_=wqT.rearrange("(s p) o -> p s o", p=P)
            )
            nc.gpsimd.dma_start(out=lnq_g_sb, in_=lnq_g.ap())
            nc.gpsimd.dma_start(out=lnq_b_sb, in_=lnq_b.ap())
            nc.gpsimd.dma_start(out=lnk_g_sb, in_=lnk_g.ap())
            nc.gpsimd.dma_start(out=lnfw_sb, in_=lnf.ap()[0:1, :])
            nc.gpsimd.dma_start(out=lnfb_sb, in_=lnf.ap()[1:2, :])

            # ---- V: 16 chunks, straight into vaug ----
            for c in range(MCH):
                py = proj_tile(xvT_sb, wv_sb, 2 * D, c)
                negmr, rstd = ln_stats(py, stat)
                nc.scalar.activation(
                    out=vaug_sb[:, c, :, 0:DH],
                    in_=py.rearrange("p (h d) -> p h d", h=H),
                    func=ACTF.Identity, bias=negmr, scale=rstd,
                )

            # ---- Q: 4 chunks ----
            qnats = []
            for c in range(NCH):
                py = proj_tile(xqT_sb, wq_sb, 0, c)
                negmr, rstd = ln_stats(py, stat)
                qnat = qn.tile([P, D], f32, tag=f"qnat{c}")
                nc.scalar.activation(
                    out=qnat, in_=py, func=ACTF.Identity, bias=negmr, scale=rstd
                )
                qnats.append(qnat)
            for s in range(G6):
                pt = ps_t.tile([P, NCH, P], f32, tag="pt")
                for c in range(NCH):
                    nc.tensor.transpose(
                        pt[:, c], qnats[c][:, s * P:(s + 1) * P], ident_f32
                    )
                nc.vector.tensor_scalar(
                    out=qT_sb[:, s, :],
                    in0=pt.rearrange("p c n -> p (c n)"),
                    scalar1=lnq_g_sb[:, s:s + 1],
                    scalar2=lnq_b_sb[:, s:s + 1],
                    op0=ALU.mult, op1=ALU.add,
                )

        # ============ phase K + attention (interleaved) ============
        with (
            tc.tile_pool(name="kp", bufs=1) as kp,
            tc.tile_pool(name="wrot2", bufs=1) as wrot2,
            tc.tile_pool(name="post", bufs=2) as postp,
            tc.tile_pool(name="stat2", bufs=6) as stat2,
            tc.tile_pool(name="attn", bufs=3) as apool,
            tc.tile_pool(name="hwork", bufs=4) as hwork,
        ):
            wk_sb = wrot2.tile([P, G6, D], bf16)
            nc.sync.dma_start(
                out=wk_sb,
                in_=wkvT.rearrange("(s p) o -> p s o", p=P)[:, :, 0:D],
            )
            knat_sb = kp.tile([P, MCH, D], bf16)

            with tc.tile_pool(name="ps_pk", bufs=3, space="PSUM") as ps_pk:
                for c in range(MCH):
                    py = ps_pk.tile([P, D], f32, tag="pyk")
                    for o0, o1 in HALVES:
                        for s in range(G6):
                            nc.tensor.matmul(
                                py[:, o0:o1],
                                xvT_sb[:, s, c * P:(c + 1) * P],
                                wk_sb[:, s, o0:o1],
                                start=(s == 0),
                                stop=(not has_qkv_bias and s == G6 - 1),
                            )
                        if has_qkv_bias:
                            nc.tensor.matmul(
                                py[:, o0:o1], ones_r, bqkv_sb[:, D + o0:D + o1],
                                start=False, stop=True,
                            )
                    negmr, rstd = ln_stats(py, stat2)
                    nc.scalar.activation(
                        out=knat_sb[:, c, :], in_=py, func=ACTF.Identity,
                        bias=negmr, scale=rstd,
                    )

            attn_psum = ExitStack()
            ps_s = attn_psum.enter_context(
                tc.tile_pool(name="ps_s", bufs=2, space="PSUM")
            )
            ps_o = attn_psum.enter_context(
                tc.tile_pool(name="ps_o", bufs=2, space="PSUM")
            )

            def build_kt_block(s):
                posT_s = postp.tile([P, N], bf16, tag="posT", name="posT_s")
                nc.sync.dma_start(
                    out=posT_s,
                    in_=posTb.rearrange("(s p) n -> p s n", p=P)[:, s, :],
                )
                for half in range(2):
                    pt = ps_s.tile([P, MCH // 2, P], bf16, tag="ps3", name="pt")
                    for c in range(MCH // 2):
                        mc = half * 8 + c
                        nc.tensor.transpose(
                            pt[:, c], knat_sb[:, mc, s * P:(s + 1) * P], ident_bf
                        )
                    nc.vector.scalar_tensor_tensor(
                        out=kT_sb[:, s, half * 1024:(half + 1) * 1024],
                        in0=pt.rearrange("p c n -> p (c n)"),
                        scalar=lnk_g_sb[:, s:s + 1],
                        in1=posT_s[:, half * 1024:(half + 1) * 1024],
                        op0=ALU.mult, op1=ALU.add,
                    )

            def head(h):
                s, p0 = h // 2, DH * (h % 2)
                po = ps_o.tile([DH + 1, S], f32, tag="po", name="po")
                for mc0, w in ((0, 3), (3, 3), (6, 3), (9, 3), (12, 3), (15, 1)):
                    ps = ps_s.tile([P, 3, S], f32, tag="ps3", name="ps")
                    for j in range(w):
                        mc = mc0 + j
                        nc.tensor.matmul(
                            ps[:, j],
                            kT_sb[p0:p0 + DH, s, mc * P:(mc + 1) * P],
                            qT_sb[p0:p0 + DH, s, :],
                            start=True, stop=True,
                        )
                    at = apool.tile([P, 3, S], bf16, tag="at", name="at")
                    nc.scalar.activation(
                        out=at[:, :w], in_=ps[:, :w], func=ACTF.Exp, scale=SCALE
                    )
                    for j in range(w):
                        mc = mc0 + j
                        nc.tensor.matmul(
                            po, vaug_sb[:, mc, h, :], at[:, j],
                            start=(mc == 0), stop=(mc == MCH - 1),
                        )
                rinv = hwork.tile([1, S], f32, tag="rinv", name="rinv")
                nc.vector.reciprocal(out=rinv, in_=po[DH:DH + 1, :])
                rbc = hwork.tile([DH, S], f32, tag="rbc", name="rbc")
                nc.gpsimd.partition_broadcast(rbc, rinv)
                nc.vector.tensor_tensor(
                    out=outT_sb[p0:p0 + DH, s, :], in0=po[0:DH, :],
                    in1=rbc, op=ALU.mult,
                )

            # software-pipelined: kT block s+1 is built between the two heads
            # of block s so its transposes/stt hide under the exp-bound heads
            build_kt_block(0)
            for s in range(G6):
                head(2 * s)
                if s + 1 < G6:
                    build_kt_block(s + 1)
                head(2 * s + 1)
            attn_psum.close()

        # ========== phase Z: out-proj, gate, fuse, final LN ==========
        with (
            tc.tile_pool(name="zw", bufs=1) as zw,
            tc.tile_pool(name="zs", bufs=2) as zs,
            tc.tile_pool(name="stat3", bufs=4) as stat3,
            tc.tile_pool(name="ps_z", bufs=2, space="PSUM") as ps_z,
            tc.tile_pool(name="ps_g", bufs=2, space="PSUM") as ps_g,
        ):
            woT_sb = zw.tile([P, G6, D], bf16)
            nc.sync.dma_start(out=woT_sb, in_=woT.rearrange("(s p) o -> p s o", p=P))
            gwvT_sb = zw.tile([P, G6, D], bf16)
            nc.scalar.dma_start(
                out=gwvT_sb, in_=gwvT.rearrange("(s p) o -> p s o", p=P)
            )
            gwzT_sb = zw.tile([P, G6, D], bf16)
            nc.gpsimd.dma_start(
                out=gwzT_sb, in_=gwzT.rearrange("(s p) o -> p s o", p=P)
            )
            bo_sb = gb_sb = None
            if has_o_bias:
                bo_sb = zw.tile([1, D], bf16)
                nc.sync.dma_start(out=bo_sb, in_=bo_a.ap())
            if has_g_bias:
                gb_sb = zw.tile([1, D], bf16)
                nc.sync.dma_start(out=gb_sb, in_=gb_e.ap())
            gbc = bbc = None
            if has_f_affine:
                gbc = zw.tile([P, D], f32)
                bbc = zw.tile([P, D], f32)
                for dst, src_row in ((gbc, lnfw_sb), (bbc, lnfb_sb)):
                    pb = ps_z.tile([P, D], f32, tag="pz")
                    for o0, o1 in HALVES:
                        nc.tensor.matmul(
                            pb[:, o0:o1], ones_r_f, src_row[:, o0:o1],
                            start=True, stop=True,
                        )
                    nc.vector.tensor_copy(out=dst, in_=pb)

            # loop A (Sigmoid/Copy table): gate + z + fuse per chunk
            fusses = []
            for c in range(NCH):
                vis_c = zs.tile([P, D], f32, tag="vis", name="vis_c")
                nc.sync.dma_start(
                    out=vis_c, in_=vis_nat.rearrange("(c p) o -> p c o", p=P)[:, c]
                )

                # gate: vis-half (own xv columns are [0, 512)) + folded z-half
                pg = ps_g.tile([P, D], f32, tag="pg")
                for o0, o1 in HALVES:
                    for s in range(G6):
                        nc.tensor.matmul(
                            pg[:, o0:o1],
                            xvT_sb[:, s, c * P:(c + 1) * P],
                            gwvT_sb[:, s, o0:o1],
                            start=(s == 0), stop=False,
                        )
                    for s in range(G6):
                        last = (not has_g_bias) and s == G6 - 1
                        nc.tensor.matmul(
                            pg[:, o0:o1],
                            outT_sb[:, s, c * P:(c + 1) * P],
                            gwzT_sb[:, s, o0:o1],
                            start=False, stop=last,
                        )
                    if has_g_bias:
                        nc.tensor.matmul(
                            pg[:, o0:o1], ones_b, gb_sb[:, o0:o1],
                            start=False, stop=True,
                        )
                gsig = zs.tile([P, D], bf16, tag="gsig", name="gsig")
                nc.scalar.activation(out=gsig, in_=pg, func=ACTF.Sigmoid)

                # out-proj z
                pz = ps_z.tile([P, D], f32, tag="pz")
                for o0, o1 in HALVES:
                    for s in range(G6):
                        last = (not has_o_bias) and s == G6 - 1
                        nc.tensor.matmul(
                            pz[:, o0:o1],
                            outT_sb[:, s, c * P:(c + 1) * P],
                            woT_sb[:, s, o0:o1],
                            start=(s == 0), stop=last,
                        )
                    if has_o_bias:
                        nc.tensor.matmul(
                            pz[:, o0:o1], ones_b, bo_sb[:, o0:o1],
                            start=False, stop=True,
                        )
                z_c = zs.tile([P, D], f32, tag="zc", name="z_c")
                nc.scalar.copy(out=z_c, in_=pz)

                # fuse: z + g*(vis - z)
                dvz = zs.tile([P, D], f32, tag="dvz", name="dvz")
                nc.gpsimd.tensor_tensor(out=dvz, in0=vis_c, in1=z_c, op=ALU.subtract)
                fus = zs.tile([P, D], f32, tag=f"fus{c}", name="fus")
                nc.vector.tensor_tensor(out=fus, in0=gsig, in1=dvz, op=ALU.mult)
                nc.vector.tensor_tensor(out=fus, in0=fus, in1=z_c, op=ALU.add)
                fusses.append(fus)

            # loop B (Sqrt/Identity table): final LayerNorm + store
            for c in range(NCH):
                fus = fusses[c]
                negmr, rstd = ln_stats(fus, stat3)
                tnorm = zs.tile([P, D], f32, tag="tnorm", name="tnorm")
                nc.scalar.activation(
                    out=tnorm, in_=fus, func=ACTF.Identity, bias=negmr, scale=rstd
                )
                if has_f_affine:
                    nc.vector.tensor_tensor(out=tnorm, in0=tnorm, in1=gbc, op=ALU.mult)
                    nc.vector.tensor_tensor(out=tnorm, in0=tnorm, in1=bbc, op=ALU.add)
                nc.sync.dma_start(
                    out=out_rows.rearrange("(c p) o -> p c o", p=P)[:, c], in_=tnorm
                )

    nc.compile()
    return nc


def _prepare_in_maps(inputs):
    f32 = np.float32
    vis = np.asarray(inputs["visible_features"], f32)
    inf = np.asarray(inputs["infrared_features"], f32)
    wq = np.asarray(inputs["wq"], f32)
    bq = np.asarray(inputs["bq"], f32)
    lnq_w = np.asarray(inputs["lnq_w"], f32)
    lnq_b = np.asarray(inputs["lnq_b"], f32)
    wk = np.asarray(inputs["wk"], f32)
    bk = np.asarray(inputs["bk"], f32)
    lnk_w = np.asarray(inputs["lnk_w"], f32)
    lnk_b = np.asarray(inputs["lnk_b"], f32)
    wv = np.asarray(inputs["wv"], f32)
    bv = np.asarray(inputs["bv"], f32)
    lnv_w = np.asarray(inputs["lnv_w"], f32)
    lnv_b = np.asarray(inputs["lnv_b"], f32)
    pos = np.asarray(inputs["pos_emb"], f32)[:N]
    wo = np.asarray(inputs["wo"], f32)
    bo = np.asarray(inputs["bo"], f32)
    gw = np.asarray(inputs["gate_w"], f32)
    gb_ = np.asarray(inputs["gate_b"], f32)
    ln_w = np.asarray(inputs["ln_w"], f32)
    ln_b = np.asarray(inputs["ln_b"], f32)

    wo_eff = wo * lnv_w[None, :]
    bo_a = bo + wo @ lnv_b
    gwv = gw[:, :D]
    gwz = gw[:, D:]
    W_eff = (gwz.astype(np.float64) @ wo_eff.astype(np.float64)).astype(f32)
    gb_eff = gb_ + gwz @ bo_a

    wqT = np.ascontiguousarray(wq.T)
    wkvT = np.ascontiguousarray(np.concatenate([wk.T, wv.T], axis=1)).astype(BF)
    woTb = np.ascontiguousarray(wo_eff.T).astype(BF)
    gwvTb = np.ascontiguousarray(gwv.T).astype(BF)
    gwzTb = np.ascontiguousarray(W_eff.T).astype(BF)
    bqkv = np.ascontiguousarray(np.concatenate([bq, bk, bv])[None])
    bo_ab = np.ascontiguousarray(bo_a[None]).astype(BF)
    gb_eb = np.ascontiguousarray(gb_eff[None]).astype(BF)
    lnq_g2 = np.ascontiguousarray(lnq_w.reshape(G6, P).T)
    lnq_b2 = np.ascontiguousarray(lnq_b.reshape(G6, P).T)
    lnk_g2 = np.ascontiguousarray(lnk_w.reshape(G6, P).T)
    lnf = np.stack([ln_w, ln_b])

    flags = (
        bool(np.any(bq) or np.any(bk) or np.any(bv)),
        bool(np.any(bo_a)),
        bool(np.any(gb_eff)),
        bool(np.any(ln_b) or np.any(ln_w != 1.0)),
    )

    posT_base = pos.T / SCALE + lnk_b[:, None]   # [D, N]

    in_maps = []
    for c in range(CORES):
        b, r0 = c // GROUP, (c % GROUP) * S
        perm = np.concatenate(
            [np.arange(r0, r0 + S), np.arange(0, r0), np.arange(r0 + S, N)]
        )
        in_maps.append({
            "xqT": np.ascontiguousarray(inf[b, r0:r0 + S].T),
            "xvT": np.ascontiguousarray(vis[b][perm].T).astype(BF),
            "vis_nat": np.ascontiguousarray(vis[b, r0:r0 + S]),
            "posTb": np.ascontiguousarray(posT_base[:, perm]).astype(BF),
            "wqT": wqT,
            "wkvT": wkvT,
            "woT": woTb,
            "gwvT": gwvTb,
            "gwzT": gwzTb,
            "lnq_g": lnq_g2,
            "lnq_b": lnq_b2,
            "lnk_g": lnk_g2,
            "bqkv": bqkv,
            "bo_a": bo_ab,
            "gb_e": gb_eb,
            "lnf": lnf,
        })
    return in_maps, flags


def kernel(trace=False, **inputs):
    from concourse.bass_utils import run_bass_kernel_spmd

    in_maps, flags = _prepare_in_maps(inputs)
    key = ("nc",) + flags
    if key not in _CACHE:
        _CACHE[key] = _build(*flags)
    nc = _CACHE[key]
    res = run_bass_kernel_spmd(
        nc, in_maps, core_ids=list(range(CORES)), trace=trace
    )
    out = np.empty((B, N, D), np.float32)
    for c in range(CORES):
        b, r0 = c // GROUP, (c % GROUP) * S
        out[b, r0:r0 + S] = res.results[c]["out_rows"]
    _CACHE["last_result"] = res
    _CACHE["nc"] = nc
    return out


# revision 22
# speedup vs baseline: 2.0912x; 1.0631x over previous
"""Trainium2 Bass kernel for nn_CustomCrossModalAttention (B=2, N=2048, D=768, H=12).

Sharding (8 cores, collective-free):
  - core c owns batch b = c//4 and query rows [512*(c%4), 512*(c%4)+512).
  - k'/v are computed REDUNDANTLY for the whole batch on each of its 4 cores
    (~45us extra PE) instead of exchanging shards: the AllGather pair cost far
    more than the replicated matmuls and serialized the whole pipeline.
  - Keys are column-PERMUTED per core so the core's own 512 rows come first
    (softmax sums over all keys, so key order is irrelevant); this makes the
    SPMD program uniform while the gate still reads "own" xv columns at a
    fixed offset 0.

Math folds (exact):
  - scores = (q@k^T)*scale + q@pos^T == scale * (q @ (LNk*g + lnk_b + pos/scale)^T)
  - LN_v gain/bias folded into wo/bo.
  - gate z-half folded through the output projection: gate = sigmoid(
      vis@gwv^T + attnout@(gwz@wo_eff)^T + gb + gwz@bo_a), removing the
    z -> zT transposes and the serialization on z.
  - All additive biases in this problem are structurally zero
    (setup_inputs uses jnp.zeros); nonzero biases are supported via
    ones-row matmuls compiled on demand (flags in the build cache key).

Dtypes (validated by numpy emulation to rel-err ~8e-3, same as the old
AllGather kernel): q path f32r end-to-end (q errors multiply the large q@pos
term in the exp argument, so bf16 there would cost ~2% at-error); k/v/gate/out
paths bf16; kT kept f32 (magnitude ~8 after the pos fold); exp/softmax in
bf16; all matmul accumulation f32 in PSUM.

Schedule: V-proj -> Q-proj -> K-proj -> per-head-pair [kT transpose block ->
attention heads 2s,2s+1] -> out-proj/gate/fuse/final-LN. Interleaving the kT
blocks with attention lets the Activation engine's softmax exp (~95us, the
2nd-busiest engine) start while PE is still projecting.
"""

import numpy as np
import ml_dtypes

B, N, D = 2, 2048, 768
H, DH = 12, 64
P = 128
CORES, GROUP = 8, 4
S = 512            # query rows per core
NCH = S // P       # 4 row chunks per core
MCH = N // P       # 16 key chunks
G6 = D // P        # 6
SCALE = DH ** -0.5
EPS = 1e-5

BF = ml_dtypes.bfloat16

_CACHE = {}

HALVES = [(0, 512), (512, D)]


def _build(has_qkv_bias, has_o_bias, has_g_bias, has_f_affine):
    from contextlib import ExitStack

    import concourse.bacc as bacc
    import concourse.mybir as mybir
    import concourse.tile as tile
    from concourse.masks import make_identity

    f32 = mybir.dt.float32
    f32r = mybir.dt.float32r
    bf16 = mybir.dt.bfloat16
    ALU = mybir.AluOpType
    ACTF = mybir.ActivationFunctionType

    nc = bacc.Bacc("TRN2", target_bir_lowering=False, num_devices=CORES)

    def din(name, shape, dt=bf16):
        return nc.dram_tensor(name, shape, dt, kind="ExternalInput")

    xqT = din("xqT", [D, S], f32r)        # own infrared rows, transposed
    xvT = din("xvT", [D, N], bf16)        # full-batch visible, transposed, key-permuted
    vis_nat = din("vis_nat", [S, D], f32)  # own visible rows, natural
    posTb = din("posTb", [D, N], bf16)    # pos/scale + lnk_b, transposed, permuted
    wqT = din("wqT", [D, D], f32r)
    wkvT = din("wkvT", [D, 2 * D], bf16)  # [wk.T | wv.T]
    woT = din("woT", [D, D], bf16)        # (wo * lnv_w).T
    gwvT = din("gwvT", [D, D], bf16)      # gate vis-half weights, transposed
    gwzT = din("gwzT", [D, D], bf16)      # (gwz @ wo_eff).T
    lnq_g = din("lnq_g", [P, G6], f32)
    lnq_b = din("lnq_b", [P, G6], f32)
    lnk_g = din("lnk_g", [P, G6], f32)
    bqkv = din("bqkv", [1, 3 * D], f32r)  # only read when has_qkv_bias
    bo_a = din("bo_a", [1, D], bf16)      # bo + wo@lnv_b
    gb_e = din("gb_e", [1, D], bf16)      # gate_b + gwz@bo_a
    lnf = din("lnf", [2, D], f32)
    out_rows = nc.dram_tensor("out_rows", [S, D], f32, kind="ExternalOutput")

    with tile.TileContext(nc) as tc, ExitStack() as ctx:
        const = ctx.enter_context(tc.tile_pool(name="const", bufs=1))
        persist = ctx.enter_context(tc.tile_pool(name="persist", bufs=1))

        ident_bf = const.tile([P, P], bf16)
        make_identity(nc, ident_bf)
        ident_f32 = const.tile([P, P], f32)
        make_identity(nc, ident_f32)
        ones_r_f = const.tile([1, P], f32)
        nc.vector.memset(ones_r_f, 1.0)
        ones_r = ones_r_f.bitcast(f32r)
        ones_b = const.tile([1, P], bf16)
        nc.vector.memset(ones_b, 1.0)
        eps_t = const.tile([P, 1], f32)
        nc.vector.memset(eps_t, EPS)

        # ---- persistent tiles ----
        xvT_sb = persist.tile([P, G6, N], bf16)
        kT_sb = persist.tile([P, G6, N], f32r)
        vaug_sb = persist.tile([P, MCH, H, DH + 1], bf16)
        qT_sb = persist.tile([P, G6, S], f32r)
        outT_sb = persist.tile([P, G6, S], bf16)
        lnq_g_sb = persist.tile([P, G6], f32)
        lnq_b_sb = persist.tile([P, G6], f32)
        lnk_g_sb = persist.tile([P, G6], f32)
        lnfw_sb = persist.tile([1, D], f32)
        lnfb_sb = persist.tile([1, D], f32)

        nc.vector.memset(vaug_sb[:, :, :, DH:DH + 1], 1.0)

        def ln_stats(y, pool):
            st = pool.tile([P, 2, 6], f32, tag="st")
            for i in range(2):
                nc.vector.bn_stats(out=st[:, i], in_=y[:, i * 384:(i + 1) * 384])
            mv = pool.tile([P, 2], f32, tag="mv")
            nc.vector.bn_aggr(out=mv, in_=st)
            rstd = pool.tile([P, 1], f32, tag="rstd")
            nc.scalar.activation(
                out=rstd, in_=mv[:, 1:2], func=ACTF.Sqrt, bias=eps_t, scale=1.0
            )
            nc.vector.reciprocal(out=rstd, in_=rstd)
            negmr = pool.tile([P, 1], f32, tag="negmr")
            nc.vector.tensor_scalar(
                out=negmr, in0=mv[:, 0:1], scalar1=rstd, scalar2=-1.0,
                op0=ALU.mult, op1=ALU.mult,
            )
            return negmr, rstd

        # ================= phase V + Q (scoped) =================
        with (
            tc.tile_pool(name="pvq", bufs=1) as pvq,
            tc.tile_pool(name="wrot", bufs=2) as wrot,
            tc.tile_pool(name="stat", bufs=6) as stat,
            tc.tile_pool(name="qn", bufs=1) as qn,
            tc.tile_pool(name="ps_p", bufs=3, space="PSUM") as ps_p,
            tc.tile_pool(name="ps_t", bufs=2, space="PSUM") as ps_t,
        ):
            # DMA issue order == need order: xvT chunk 0 and wv first (V-proj
            # starts on them), remaining xvT chunks, then the q-path tensors.
            nc.sync.dma_start(
                out=xvT_sb[:, :, 0:S],
                in_=xvT.rearrange("(s p) n -> p s n", p=P)[:, :, 0:S],
            )
            wv_sb = wrot.tile([P, G6, D], bf16, tag="wkv")
            nc.scalar.dma_start(
                out=wv_sb,
                in_=wkvT.rearrange("(s p) o -> p s o", p=P)[:, :, D:2 * D],
            )
            for i in range(1, 4):
                nc.gpsimd.dma_start(
                    out=xvT_sb[:, :, i * S:(i + 1) * S],
                    in_=xvT.rearrange("(s p) n -> p s n", p=P)[:, :, i * S:(i + 1) * S],
                )
            bqkv_sb = None
            if has_qkv_bias:
                bqkv_sb = pvq.tile([1, 3 * D], f32r)
                nc.sync.dma_start(out=bqkv_sb, in_=bqkv.ap())

            def proj_tile(lhsT_sb, w_sb, w_off, c):
                py = ps_p.tile([P, D], f32, tag="py")
                for o0, o1 in HALVES:
                    for s in range(G6):
                        nc.tensor.matmul(
                            py[:, o0:o1],
                            lhsT_sb[:, s, c * P:(c + 1) * P],
                            w_sb[:, s, o0:o1],
                            start=(s == 0), stop=(not has_qkv_bias and s == G6 - 1),
                        )
                    if has_qkv_bias:
                        nc.tensor.matmul(
                            py[:, o0:o1], ones_r,
                            bqkv_sb[:, w_off + o0:w_off + o1],
                            start=False, stop=True,
                        )
                return py

            xqT_sb = pvq.tile([P, G6, S], f32r)
            nc.scalar.dma_start(
                out=xqT_sb, in_=xqT.rearrange("(s p) n -> p s n", p=P)
            )
            wq_sb = pvq.tile([P, G6, D], f32r)
            nc.sync.dma_start(
                out=wq_sb, in_=wqT.rearrange("(s p) o -> p s o", p=P)
            )
            nc.gpsimd.dma_start(out=lnq_g_sb, in_=lnq_g.ap())
            nc.gpsimd.dma_start(out=lnq_b_sb, in_=lnq_b.ap())
            nc.gpsimd.dma_start(out=lnk_g_sb, in_=lnk_g.ap())
            nc.gpsimd.dma_start(out=lnfw_sb, in_=lnf.ap()[0:1, :])
            nc.gpsimd.dma_start(out=lnfb_sb, in_=lnf.ap()[1:2, :])

            # ---- V: 16 chunks, straight into vaug ----
            for c in range(MCH):
                py = proj_tile(xvT_sb, wv_sb, 2 * D, c)
                negmr, rstd = ln_stats(py, stat)
                nc.scalar.activation(
                    out=vaug_sb[:, c, :, 0:DH],
                    in_=py.rearrange("p (h d) -> p h d", h=H),
                    func=ACTF.Identity, bias=negmr, scale=rstd,
                )

            # ---- Q: 4 chunks ----
            qnats = []
            for c in range(NCH):
                py = proj_tile(xqT_sb, wq_sb, 0, c)
                negmr, rstd = ln_stats(py, stat)
                qnat = qn.tile([P, D], f32, tag=f"qnat{c}")
                nc.scalar.activation(
                    out=qnat, in_=py, func=ACTF.Identity, bias=negmr, scale=rstd
                )
                qnats.append(qnat)
            for s in range(G6):
                pt = ps_t.tile([P, NCH, P], f32, tag="pt")
                for c in range(NCH):
                    nc.tensor.transpose(
                        pt[:, c], qnats[c][:, s * P:(s + 1) * P], ident_f32
                    )
                nc.vector.tensor_scalar(
                    out=qT_sb[:, s, :],
                    in0=pt.rearrange("p c n -> p (c n)"),
                    scalar1=lnq_g_sb[:, s:s + 1],
                    scalar2=lnq_b_sb[:, s:s + 1],
                    op0=ALU.mult, op1=ALU.add,
                )

        # ============ phase K + attention (interleaved) ============
        with (
            tc.tile_pool(name="kp", bufs=1) as kp,
            tc.tile_pool(name="wrot2", bufs=1) as wrot2,
            tc.tile_pool(name="post", bufs=2) as postp,
            tc.tile_pool(name="stat2", bufs=6) as stat2,
            tc.tile_pool(name="attn", bufs=3) as apool,
            tc.tile_pool(name="hwork", bufs=4) as hwork,
        ):
            wk_sb = wrot2.tile([P, G6, D], bf16)
            nc.sync.dma_start(
                out=wk_sb,
                in_=wkvT.rearrange("(s p) o -> p s o", p=P)[:, :, 0:D],
            )
            knat_sb = kp.tile([P, MCH, D], bf16)

            with tc.tile_pool(name="ps_pk", bufs=3, space="PSUM") as ps_pk:
                for c in range(MCH):
                    py = ps_pk.tile([P, D], f32, tag="pyk")
                    for o0, o1 in HALVES:
                        for s in range(G6):
                            nc.tensor.matmul(
                                py[:, o0:o1],
                                xvT_sb[:, s, c * P:(c + 1) * P],
                                wk_sb[:, s, o0:o1],
                                start=(s == 0),
                                stop=(not has_qkv_bias and s == G6 - 1),
                            )
                        if has_qkv_bias:
                            nc.tensor.matmul(
                                py[:, o0:o1], ones_r, bqkv_sb[:, D + o0:D + o1],
                                start=False, stop=True,
                            )
                    negmr, rstd = ln_stats(py, stat2)
                    nc.scalar.activation(
                        out=knat_sb[:, c, :], in_=py, func=ACTF.Identity,
                        bias=negmr, scale=rstd,
                    )

            attn_psum = ExitStack()
            ps_s = attn_psum.enter_context(
                tc.tile_pool(name="ps_s", bufs=2, space="PSUM")
            )
            ps_kt = attn_psum.enter_context(
                tc.tile_pool(name="ps_kt", bufs=2, space="PSUM")
            )
            ps_o = attn_psum.enter_context(
                tc.tile_pool(name="ps_o", bufs=2, space="PSUM")
            )

            # preload the Exp act table while PE is still on K-proj: the
            # LN applies (Identity) work under any table, so this is the
            # only swap and it runs off the critical path
            dummy = hwork.tile([P, 1], bf16, tag="dummy", name="dummy")
            nc.scalar.activation(out=dummy, in_=eps_t, func=ACTF.Exp)

            def build_kt_block(s):
                posT_s = postp.tile([P, N], bf16, tag="posT", name="posT_s")
                nc.sync.dma_start(
                    out=posT_s,
                    in_=posTb.rearrange("(s p) n -> p s n", p=P)[:, s, :],
                )
                for half in range(2):
                    pt = ps_kt.tile([P, MCH // 2, P], bf16, tag="pt", name="pt")
                    for c in range(MCH // 2):
                        mc = half * 8 + c
                        nc.tensor.transpose(
                            pt[:, c], knat_sb[:, mc, s * P:(s + 1) * P], ident_bf
                        )
                    nc.vector.scalar_tensor_tensor(
                        out=kT_sb[:, s, half * 1024:(half + 1) * 1024],
                        in0=pt.rearrange("p c n -> p (c n)"),
                        scalar=lnk_g_sb[:, s:s + 1],
                        in1=posT_s[:, half * 1024:(half + 1) * 1024],
                        op0=ALU.mult, op1=ALU.add,
                    )

            def head(h):
                s, p0 = h // 2, DH * (h % 2)
                po = ps_o.tile([DH + 1, S], f32, tag="po", name="po")
                for mc0 in range(0, MCH, 2):
                    ps = ps_s.tile([P, 2, S], f32, tag="ps2", name="ps")
                    for j in range(2):
                        mc = mc0 + j
                        nc.tensor.matmul(
                            ps[:, j],
                            kT_sb[p0:p0 + DH, s, mc * P:(mc + 1) * P],
                            qT_sb[p0:p0 + DH, s, :],
                            start=True, stop=True,
                        )
                    at = apool.tile([P, 2, S], bf16, tag="at", name="at")
                    nc.scalar.activation(
                        out=at, in_=ps, func=ACTF.Exp, scale=SCALE
                    )
                    for j in range(2):
                        mc = mc0 + j
                        nc.tensor.matmul(
                            po, vaug_sb[:, mc, h, :], at[:, j],
                            start=(mc == 0), stop=(mc == MCH - 1),
                        )
                rinv = hwork.tile([1, S], f32, tag="rinv", name="rinv")
                nc.vector.reciprocal(out=rinv, in_=po[DH:DH + 1, :])
                rbc = hwork.tile([DH, S], f32, tag="rbc", name="rbc")
                nc.gpsimd.partition_broadcast(rbc, rinv)
                nc.vector.tensor_tensor(
                    out=outT_sb[p0:p0 + DH, s, :], in0=po[0:DH, :],
                    in1=rbc, op=ALU.mult,
                )

            # software-pipelined: kT block s+1 is built between the two heads
            # of block s so its transposes/stt hide under the exp-bound heads
            build_kt_block(0)
            for s in range(G6):
                head(2 * s)
                if s + 1 < G6:
                    build_kt_block(s + 1)
                head(2 * s + 1)
            attn_psum.close()

        # ========== phase Z: out-proj, gate, fuse, final LN ==========
        with (
            tc.tile_pool(name="zw", bufs=1) as zw,
            tc.tile_pool(name="zs", bufs=2) as zs,
            tc.tile_pool(name="stat3", bufs=4) as stat3,
            tc.tile_pool(name="ps_z", bufs=2, space="PSUM") as ps_z,
            tc.tile_pool(name="ps_g", bufs=2, space="PSUM") as ps_g,
        ):
            woT_sb = zw.tile([P, G6, D], bf16)
            nc.sync.dma_start(out=woT_sb, in_=woT.rearrange("(s p) o -> p s o", p=P))
            gwvT_sb = zw.tile([P, G6, D], bf16)
            nc.scalar.dma_start(
                out=gwvT_sb, in_=gwvT.rearrange("(s p) o -> p s o", p=P)
            )
            gwzT_sb = zw.tile([P, G6, D], bf16)
            nc.gpsimd.dma_start(
                out=gwzT_sb, in_=gwzT.rearrange("(s p) o -> p s o", p=P)
            )
            bo_sb = gb_sb = None
            if has_o_bias:
                bo_sb = zw.tile([1, D], bf16)
                nc.sync.dma_start(out=bo_sb, in_=bo_a.ap())
            if has_g_bias:
                gb_sb = zw.tile([1, D], bf16)
                nc.sync.dma_start(out=gb_sb, in_=gb_e.ap())
            gbc = bbc = None
            if has_f_affine:
                gbc = zw.tile([P, D], f32)
                bbc = zw.tile([P, D], f32)
                for dst, src_row in ((gbc, lnfw_sb), (bbc, lnfb_sb)):
                    pb = ps_z.tile([P, D], f32, tag="pz")
                    for o0, o1 in HALVES:
                        nc.tensor.matmul(
                            pb[:, o0:o1], ones_r_f, src_row[:, o0:o1],
                            start=True, stop=True,
                        )
                    nc.vector.tensor_copy(out=dst, in_=pb)

            # loop A (Sigmoid/Copy table): gate + z + fuse per chunk
            fusses = []
            for c in range(NCH):
                vis_c = zs.tile([P, D], f32, tag="vis", name="vis_c")
                nc.sync.dma_start(
                    out=vis_c, in_=vis_nat.rearrange("(c p) o -> p c o", p=P)[:, c]
                )

                # gate: vis-half (own xv columns are [0, 512)) + folded z-half
                pg = ps_g.tile([P, D], f32, tag="pg")
                for o0, o1 in HALVES:
                    for s in range(G6):
                        nc.tensor.matmul(
                            pg[:, o0:o1],
                            xvT_sb[:, s, c * P:(c + 1) * P],
                            gwvT_sb[:, s, o0:o1],
                            start=(s == 0), stop=False,
                        )
                    for s in range(G6):
                        last = (not has_g_bias) and s == G6 - 1
                        nc.tensor.matmul(
                            pg[:, o0:o1],
                            outT_sb[:, s, c * P:(c + 1) * P],
                            gwzT_sb[:, s, o0:o1],
                            start=False, stop=last,
                        )
                    if has_g_bias:
                        nc.tensor.matmul(
                            pg[:, o0:o1], ones_b, gb_sb[:, o0:o1],
                            start=False, stop=True,
                        )
                gsig = zs.tile([P, D], bf16, tag="gsig", name="gsig")
                nc.scalar.activation(out=gsig, in_=pg, func=ACTF.Sigmoid)

                # out-proj z
                pz = ps_z.tile([P, D], f32, tag="pz")
                for o0, o1 in HALVES:
                    for s in range(G6):
                        last = (not has_o_bias) and s == G6 - 1
                        nc.tensor.matmul(
                            pz[:, o0:o1],
                            outT_sb[:, s, c * P:(c + 1) * P],
                            woT_sb[:, s, o0:o1],
                            start=(s == 0), stop=last,
                        )
                    if has_o_bias:
                        nc.tensor.matmul(
                            pz[:, o0:o1], ones_b, bo_sb[:, o0:o1],
                            start=False, stop=True,
                        )
                z_c = zs.tile([P, D], f32, tag="zc", name="z_c")
                nc.scalar.copy(out=z_c, in_=pz)

                # fuse: z + g*(vis - z)
                dvz = zs.tile([P, D], f32, tag="dvz", name="dvz")
                nc.gpsimd.tensor_tensor(out=dvz, in0=vis_c, in1=z_c, op=ALU.subtract)
                fus = zs.tile([P, D], f32, tag=f"fus{c}", name="fus")
                nc.vector.tensor_tensor(out=fus, in0=gsig, in1=dvz, op=ALU.mult)
                nc.vector.tensor_tensor(out=fus, in0=fus, in1=z_c, op=ALU.add)
                fusses.append(fus)

            # loop B (Sqrt/Identity table): final LayerNorm + store
            for c in range(NCH):
                fus = fusses[c]
                negmr, rstd = ln_stats(fus, stat3)
                tnorm = zs.tile([P, D], f32, tag="tnorm", name="tnorm")
                nc.scalar.activation(
                    out=tnorm, in_=fus, func=ACTF.Identity, bias=negmr, scale=rstd
                )
                if has_f_affine:
                    nc.vector.tensor_tensor(out=tnorm, in0=tnorm, in1=gbc, op=ALU.mult)
                    nc.vector.tensor_tensor(out=tnorm, in0=tnorm, in1=bbc, op=ALU.add)
                nc.sync.dma_start(
                    out=out_rows.rearrange("(c p) o -> p c o", p=P)[:, c], in_=tnorm
                )

    nc.compile()
    return nc


def _prepare_in_maps(inputs):
    f32 = np.float32
    vis = np.asarray(inputs["visible_features"], f32)
    inf = np.asarray(inputs["infrared_features"], f32)
    wq = np.asarray(inputs["wq"], f32)
    bq = np.asarray(inputs["bq"], f32)
    lnq_w = np.asarray(inputs["lnq_w"], f32)
    lnq_b = np.asarray(inputs["lnq_b"], f32)
    wk = np.asarray(inputs["wk"], f32)
    bk = np.asarray(inputs["bk"], f32)
    lnk_w = np.asarray(inputs["lnk_w"], f32)
    lnk_b = np.asarray(inputs["lnk_b"], f32)
    wv = np.asarray(inputs["wv"], f32)
    bv = np.asarray(inputs["bv"], f32)
    lnv_w = np.asarray(inputs["lnv_w"], f32)
    lnv_b = np.asarray(inputs["lnv_b"], f32)
    pos = np.asarray(inputs["pos_emb"], f32)[:N]
    wo = np.asarray(inputs["wo"], f32)
    bo = np.asarray(inputs["bo"], f32)
    gw = np.asarray(inputs["gate_w"], f32)
    gb_ = np.asarray(inputs["gate_b"], f32)
    ln_w = np.asarray(inputs["ln_w"], f32)
    ln_b = np.asarray(inputs["ln_b"], f32)

    wo_eff = wo * lnv_w[None, :]
    bo_a = bo + wo @ lnv_b
    gwv = gw[:, :D]
    gwz = gw[:, D:]
    W_eff = (gwz.astype(np.float64) @ wo_eff.astype(np.float64)).astype(f32)
    gb_eff = gb_ + gwz @ bo_a

    wqT = np.ascontiguousarray(wq.T)
    wkvT = np.ascontiguousarray(np.concatenate([wk.T, wv.T], axis=1)).astype(BF)
    woTb = np.ascontiguousarray(wo_eff.T).astype(BF)
    gwvTb = np.ascontiguousarray(gwv.T).astype(BF)
    gwzTb = np.ascontiguousarray(W_eff.T).astype(BF)
    bqkv = np.ascontiguousarray(np.concatenate([bq, bk, bv])[None])
    bo_ab = np.ascontiguousarray(bo_a[None]).astype(BF)
    gb_eb = np.ascontiguousarray(gb_eff[None]).astype(BF)
    lnq_g2 = np.ascontiguousarray(lnq_w.reshape(G6, P).T)
    lnq_b2 = np.ascontiguousarray(lnq_b.reshape(G6, P).T)
    lnk_g2 = np.ascontiguousarray(lnk_w.reshape(G6, P).T)
    lnf = np.stack([ln_w, ln_b])

    flags = (
        bool(np.any(bq) or np.any(bk) or np.any(bv)),
        bool(np.any(bo_a)),
        bool(np.any(gb_eff)),
        bool(np.any(ln_b) or np.any(ln_w != 1.0)),
    )

    posT_base = pos.T / SCALE + lnk_b[:, None]   # [D, N]

    in_maps = []
    for c in range(CORES):
        b, r0 = c // GROUP, (c % GROUP) * S
        perm = np.concatenate(
            [np.arange(r0, r0 + S), np.arange(0, r0), np.arange(r0 + S, N)]
        )
        in_maps.append({
            "xqT": np.ascontiguousarray(inf[b, r0:r0 + S].T),
            "xvT": np.ascontiguousarray(vis[b][perm].T).astype(BF),
            "vis_nat": np.ascontiguousarray(vis[b, r0:r0 + S]),
            "posTb": np.ascontiguousarray(posT_base[:, perm]).astype(BF),
            "wqT": wqT,
            "wkvT": wkvT,
            "woT": woTb,
            "gwvT": gwvTb,
            "gwzT": gwzTb,
            "lnq_g": lnq_g2,
            "lnq_b": lnq_b2,
            "lnk_g": lnk_g2,
            "bqkv": bqkv,
            "bo_a": bo_ab,
            "gb_e": gb_eb,
            "lnf": lnf,
        })
    return in_maps, flags


def kernel(trace=False, **inputs):
    from concourse.bass_utils import run_bass_kernel_spmd

    in_maps, flags = _prepare_in_maps(inputs)
    key = ("nc",) + flags
    if key not in _CACHE:
        _CACHE[key] = _build(*flags)
    nc = _CACHE[key]
    res = run_bass_kernel_spmd(
        nc, in_maps, core_ids=list(range(CORES)), trace=trace
    )
    out = np.empty((B, N, D), np.float32)
    for c in range(CORES):
        b, r0 = c // GROUP, (c % GROUP) * S
        out[b, r0:r0 + S] = res.results[c]["out_rows"]
    _CACHE["last_result"] = res
    _CACHE["nc"] = nc
    return out
